# revision 1
# baseline (speedup 1.0000x reference)
"""Trainium2 Bass kernel for nn_BiasedMHABlock (biased MHA + FFN transformer block).

Sharding: batch B=8 -> one batch per NeuronCore (SPMD, no collectives).

Per-core math (batch b), fully fused on-device:
  scores^T[w,u] per head = (K_h Q_h^T)/8 + CB[w,u] + relband_h[w,u], where
  softmax-invariant constants are dropped and
  CB = simscale*Xn Xn^T - gate*OneHot(spk) OneHot(spk)^T is built once via PE
  and added per-head with identity-matmul PSUM accumulation.
  Softmax runs over the partition axis without max-subtraction (scores are O(1));
  the denominator comes free from an appended ones-column of V in the attn@V
  matmul and is divided out post-hoc.
  Then X1 = LN(X+bo + Attn@Wo), FFN with transposed hidden, X2 = LN(X1+ffn).

All matmuls run as float32r (full-rate fp32, ~tf32 rounding on inputs,
fp32 accumulation); residual adds stay in exact fp32 on the vector engine.
"""
import sys
import math

import os
for _p in ("/opt/trn_rl_repo", "/root/.axon_site/_ro/trn_rl_repo"):
    if os.path.isdir(_p) and _p not in sys.path:
        sys.path.insert(0, _p)

import numpy as np
import ml_dtypes

import concourse.bass as bass
import concourse.tile as tile
from concourse import bacc, mybir
from concourse.bass_utils import run_bass_kernel_spmd

F32 = mybir.dt.float32
F32R = mybir.dt.float32r
BF16 = mybir.dt.bfloat16
AF = mybir.ActivationFunctionType
ALU = mybir.AluOpType

B, U, D, H, DH, DFF = 8, 1024, 512, 8, 64, 4096
REL_MAX = 128
P = 128
NCORES = 8
LN_EPS = 1e-5
UBLK = 512  # ffn u-block

_prog_cache = {}
SKIP = set()  # perf-analysis only: phase names to skip


def _build_program(fast_gates: bool, apply_mask: bool, ncat: int, ln1_triv: bool = False, ln2_triv: bool = False):
    nc = bacc.Bacc("TRN2", target_bir_lowering=False, debug=False)

    def din(name, shape, dt=F32R):
        return nc.dram_tensor(name, list(shape), dt, kind="ExternalInput").ap()

    xt = din("xt", [4, P, U])
    xpbo = din("xpbo", [8, P, D], F32)
    rns_a = din("rns_a", [1, U], F32)
    rns_b = din("rns_b", [1, U], F32)
    pta = din("pta", [H, ncat, U])
    ptb = din("ptb", [ncat, U])
    wq = din("wq", [4, P, D])
    wk = din("wk", [4, P, D])
    wv = din("wv", [5, P, D])
    wo = din("wo", [4, P, D])
    w1 = din("w1", [4, P, DFF])
    w2 = din("w2", [33, P, D])
    bf1p = din("bf1p", [P, 32], F32)
    qkb = din("qkb", [P, 8], F32)
    rbd = din("rbd", [P, H, 3, P])
    lnw = din("lnw", [4, D], F32)
    expd = din("expd", [4, 2 * P])
    uvec4 = din("uvec4", [1, 16])
    identd = din("identd", [P, P])
    identfd = din("identfd", [P, P], F32)
    ones_pe = din("ones_pe", [1, P])
    ones_v = din("ones_v", [P, 64])
    validd = din("validd", [P, 8], F32)
    if not fast_gates:
        sidents = din("sidents", [H, P, P])
        gidents = din("gidents", [H, P, P])

    out = nc.dram_tensor("out", [8, P, D], F32, kind="ExternalOutput").ap()

    open_cms = {}

    with tile.TileContext(nc) as tc, nc.allow_low_precision(reason="fp32r kernel"):
        def pool(name, bufs, space="SBUF", side="left"):
            cm = tc.tile_pool(name=name, bufs=bufs, space=space, side=side)
            p = cm.__enter__()
            open_cms[name] = cm
            return p

        def close(*names):
            for n in names:
                open_cms.pop(n).__exit__(None, None, None)

        try:
            # ---------------- constants (left, whole-kernel) ----------------
            consts = pool("consts", 1)
            ident = consts.tile([P, P], F32R)
            identf = consts.tile([P, P], F32)
            qkb_t = consts.tile([P, 8], F32)
            bf1_t = consts.tile([P, 32], F32)
            valid_t = consts.tile([P, 8], F32)
            epst = consts.tile([P, 1], F32)
            ones_u = consts.tile([1, P], F32R)
            ones_bf = consts.tile([1, P], F32R)

            # ------------- long-lived attention inputs (left) ----------------
            attn_in = pool("attn_in", 1)
            qt_t = attn_in.tile([P, 4, U], F32R, tag="qt", name="qt")
            kt_t = attn_in.tile([P, 4, U], F32R, tag="kt", name="kt")
            vt_t = attn_in.tile([P, 8, 520], F32R, tag="vt", name="vt")
            rb_t = attn_in.tile([P, H, 3, P], F32R, tag="rbt", name="rbt")
            if fast_gates:
                cb_mats = [attn_in.tile([P, 8, U], F32R, tag="cbt", name="cbt")]
            else:
                cb_mats = [
                    attn_in.tile([P, 8, U], F32R, tag="simt", name="simt"),
                    attn_in.tile([P, 8, U], F32R, tag="eqt", name="eqt"),
                ]
                sid_t = attn_in.tile([P, H, P], F32R, tag="sid", name="sid")
                nc.sync.dma_start(sid_t, sidents.rearrange("h p q -> p h q"))
                gid_t = attn_in.tile([P, H, P], F32R, tag="gid", name="gid")
                nc.sync.dma_start(gid_t, gidents.rearrange("h p q -> p h q"))

            # ======================= PHASE 1: prep ==========================
            pre = pool("pre", 1, side="right")
            prew = pool("prew", 2, side="right")
            ps1 = pool("ps1", 2, space="PSUM")

            xt_t = pre.tile([P, 4, U], F32R, tag="xt", name="xtt")
            nc.sync.dma_start(xt_t[:, 0, 0:D], xt[0][:, 0:D])
            wq_t = prew.tile([P, 5, D], F32R, tag="wx", name="wqt")
            wk_t = prew.tile([P, 5, D], F32R, tag="wx", name="wkt")
            nc.sync.dma_start(wq_t[:, 0, :], wq[0])
            for c in range(1, 4):
                nc.sync.dma_start(xt_t[:, c, 0:D], xt[c][:, 0:D])
                nc.sync.dma_start(wq_t[:, c, :], wq[c])
            for c in range(4):
                nc.sync.dma_start(xt_t[:, c, D:U], xt[c][:, D:U])
                nc.sync.dma_start(wk_t[:, c, :], wk[c])
            nc.sync.dma_start(qkb_t, qkb)
            if fast_gates:
                rnsa_b = pre.tile([P, U], F32, tag="rnsa", name="rnsa")
                nc.gpsimd.dma_start(
                    rnsa_b,
                    bass.AP(tensor=rns_a.tensor, offset=0, ap=[[0, P], [1, U]]),
                )
            rnsb_b = pre.tile([P, U], F32, tag="rnsb", name="rnsb")
            nc.gpsimd.dma_start(
                rnsb_b, bass.AP(tensor=rns_b.tensor, offset=0, ap=[[0, P], [1, U]])
            )
            nc.sync.dma_start(ident, identd)
            nc.sync.dma_start(identf, identfd)
            nc.sync.dma_start(bf1_t, bf1p)
            nc.sync.dma_start(valid_t, validd)
            nc.vector.memset(epst, LN_EPS)
            nc.sync.dma_start(ones_u, ones_pe)
            nc.sync.dma_start(ones_bf, ones_pe)

            # Q^T, K^T: psum[e_tile, u_half] = sum_c Wx[c]-slice^T @ XT
            for (wt, dst, boff) in () if "qkproj" in SKIP else ((wq_t, qt_t, 0), (wk_t, kt_t, 4)):
                for t in range(4):
                    for j in range(2):
                        ps = ps1.tile([P, D], F32, tag="psqk", name="psqk")
                        for c in range(4):
                            nc.tensor.matmul(
                                ps,
                                wt[:, c, t * P:(t + 1) * P],
                                xt_t[:, c, j * D:(j + 1) * D],
                                start=(c == 0), stop=(c == 3),
                            )
                        nc.scalar.activation(
                            dst[:, t, j * D:(j + 1) * D], ps, AF.Identity,
                            bias=qkb_t[:, boff + t:boff + t + 1],
                        )

            # V (interleaved (dh h) layout + ones cols)
            wv_t = prew.tile([P, 5, D], F32R, tag="wx", name="wvt")
            for c in range(5):
                nc.sync.dma_start(wv_t[:, c, :], wv[c])
            for t in range(0 if "vproj" in SKIP else 8):
                ps = ps1.tile([P, D], F32, tag="psv", name="psv")
                for c in range(4):
                    nc.tensor.matmul(
                        ps, xt_t[:, c, t * P:(t + 1) * P], wv_t[:, c, :],
                        start=(c == 0), stop=False,
                    )
                nc.tensor.matmul(
                    ps, ones_u[0:1, :], wv_t[0:1, 4, :],
                    start=False, stop=True,
                )
                nc.vector.tensor_copy(
                    vt_t[:, t, :].rearrange("p (h c) -> p h c", c=65)[:, :, 0:64],
                    ps.rearrange("p (h dh) -> p h dh", h=H),
                )
            nc.sync.dma_start(
                vt_t.rearrange("p t (h c) -> p t h c", c=65)[:, :, :, 64:65],
                ones_v.rearrange("p (t h o) -> p t h o", t=8, h=8),
            )
            if apply_mask:
                for t in range(8):
                    nc.vector.tensor_scalar_mul(
                        vt_t[:, t, :], vt_t[:, t, :], valid_t[:, t:t + 1],
                    )

            # Xn^T (scaled / unscaled sides) and CB (or SIM + EQ)
            if fast_gates:
                xna_t = pre.tile([P, 4, U], F32R, tag="xna", name="xna")
            xnb_t = pre.tile([P, 4, U], F32R, tag="xnb", name="xnb")
            for c in range(4):
                if fast_gates:
                    nc.vector.tensor_tensor(
                        xna_t[:, c, :], xt_t[:, c, :], rnsa_b, ALU.mult
                    )
                nc.vector.tensor_tensor(
                    xnb_t[:, c, :], xt_t[:, c, :], rnsb_b, ALU.mult
                )

            ptb_t = pre.tile([ncat, U], F32R, tag="ptb", name="ptbt")
            nc.sync.dma_start(ptb_t, ptb)
            if fast_gates:
                pta_t = pre.tile([ncat, 1, U], F32R, tag="pta", name="ptat")
                nc.sync.dma_start(pta_t[:, 0, :], pta[0])

            if fast_gates:
                cbt = cb_mats[0]
                for i in range(0 if "cb" in SKIP else 8):
                    for j in range(2):
                        ps = ps1.tile([P, D], F32, tag="pscb", name="pscb")
                        for c in range(4):
                            nc.tensor.matmul(
                                ps,
                                xna_t[:, c, i * P:(i + 1) * P],
                                xnb_t[:, c, j * D:(j + 1) * D],
                                start=(c == 0), stop=False,
                            )
                        nc.tensor.matmul(
                            ps,
                            pta_t[:, 0, i * P:(i + 1) * P],
                            ptb_t[:, j * D:(j + 1) * D],
                            start=False, stop=True,
                        )
                        nc.vector.tensor_copy(
                            cbt[:, i, j * D:(j + 1) * D], ps
                        )
            else:
                simt, eqt = cb_mats
                for i in range(8):
                    for j in range(2):
                        ps = ps1.tile([P, D], F32, tag="pscb", name="pscb")
                        for c in range(4):
                            nc.tensor.matmul(
                                ps,
                                xnb_t[:, c, i * P:(i + 1) * P],
                                xnb_t[:, c, j * D:(j + 1) * D],
                                start=(c == 0), stop=(c == 3),
                            )
                        nc.scalar.activation(
                            simt[:, i, j * D:(j + 1) * D], ps, AF.Copy
                        )
                        ps2 = ps1.tile([P, D], F32, tag="pscb", name="pscb2")
                        nc.tensor.matmul(
                            ps2,
                            ptb_t[:, i * P:(i + 1) * P],
                            ptb_t[:, j * D:(j + 1) * D],
                            start=True, stop=True,
                        )
                        nc.scalar.activation(
                            eqt[:, i, j * D:(j + 1) * D], ps2, AF.Copy
                        )

            close("ps1", "prew", "pre")

            nc.sync.dma_start(rb_t, rbd)

            # ====================== PHASE 2: attention ======================
            mid = pool("mid", 1, side="right")
            attnT = mid.tile([P, 4, U], F32R, tag="attnT", name="attnT")
            den_sb = mid.tile([1, 16, D], F32R, tag="densb", name="densb")
            expd_t = mid.tile([4, 2 * P], F32R, tag="expd", name="expdt")
            nc.sync.dma_start(expd_t, expd)
            uvec_t = mid.tile([1, 16], F32R, tag="uvec", name="uvect")
            nc.sync.dma_start(uvec_t, uvec4)

            epool = pool("epool", 4, side="right")
            dnp = pool("dnp", 4, side="right")
            pss = pool("pss", 4, space="PSUM")
            psa = pool("psa", 4, space="PSUM")

            for h in range(0 if "attn" in SKIP else H):
                po = (h % 2) * 64
                ch = h // 2
                patts = [
                    psa.tile([65, D], F32, tag="psatt", name=f"psatt_{h}_{j}")
                    for j in range(2)
                ]
                for i in range(8):
                    et = epool.tile([P, U], F32R, tag="et", name="et")
                    for j in range(2):
                        ps = pss.tile([P, D], F32, tag="pssc", name="pssc")
                        mms = [(
                            kt_t[po:po + 64, ch, i * P:(i + 1) * P],
                            qt_t[po:po + 64, ch, j * D:(j + 1) * D],
                            slice(0, D),
                        )]
                        if fast_gates:
                            adds = [(ident, cb_mats[0])]
                        else:
                            adds = [(sid_t[:, h, :], cb_mats[0]),
                                    (gid_t[:, h, :], cb_mats[1])]
                        for (idm, mat) in adds:
                            mms.append((
                                idm,
                                mat[:, i, j * D:(j + 1) * D],
                                slice(0, D),
                            ))
                        # banded rel bias: blocks i-1, i, i+1, clipped to half j
                        lo_b = max(i - 1, 0)
                        hi_b = min(i + 1, 7)
                        run_lo = max(lo_b * P, j * D)
                        run_hi = min((hi_b + 1) * P, (j + 1) * D)
                        if run_hi > run_lo:
                            o0 = (run_lo // P) - (i - 1)
                            o1 = (run_hi // P) - (i - 1)
                            mms.append((
                                ident,
                                rb_t[:, h, o0:o1, :],
                                slice(run_lo - j * D, run_hi - j * D),
                            ))
                        for mi, (lhsT, rhs, osl) in enumerate(mms):
                            nc.tensor.matmul(
                                ps[:, osl], lhsT, rhs,
                                start=(mi == 0), stop=(mi == len(mms) - 1),
                                skip_group_check=True,
                            )
                        nc.scalar.activation(
                            et[:, j * D:(j + 1) * D], ps, AF.Exp
                        )
                    for j in range(2):
                        nc.tensor.matmul(
                            patts[j],
                            vt_t[:, i, h * 65:h * 65 + 65],
                            et[:, j * D:(j + 1) * D],
                            start=(i == 0), stop=(i == 7),
                        )
                for j in range(2):
                    idx = h * 2 + j
                    nc.vector.tensor_copy(
                        den_sb[0:1, idx, :], patts[j][64:65, :]
                    )
                    nc.vector.tensor_copy(
                        attnT[po:po + 64, ch, j * D:(j + 1) * D],
                        patts[j][0:64, :],
                    )
                if h % 2 == 1:
                    # normalize chunk ch: heads 2ch, 2ch+1 are done
                    c4 = 4 * ch
                    psg = psa.tile([4, D], F32, tag="psatt", name=f"psg_{ch}")
                    for r in range(4):
                        nc.tensor.matmul(
                            psg,
                            uvec_t[0:1, r * 4:(r + 1) * 4],
                            den_sb[0:1, c4 + r, :],
                            start=(r == 0), stop=(r == 3),
                        )
                    rden4 = dnp.tile([4, D], F32R, tag="rden4", name="rden4")
                    nc.vector.reciprocal(rden4, psg)
                    for j in range(2):
                        psn = psa.tile([P, D], F32, tag="psatt", name=f"psn_{ch}_{j}")
                        nc.tensor.matmul(
                            psn,
                            expd_t[:, j * P:(j + 1) * P],
                            rden4,
                            start=True, stop=True,
                        )
                        nc.vector.tensor_tensor(
                            attnT[:, ch, j * D:(j + 1) * D],
                            attnT[:, ch, j * D:(j + 1) * D],
                            psn, ALU.mult,
                        )

            close("psa", "pss", "dnp", "epool")
            close("attn_in")

            # ---------- x1 pool opens early on the left (outlives mid) -------
            x1p = pool("x1p", 1)
            x1_t = x1p.tile([P, 8, D], F32, tag="x1", name="x1")
            x1T_t = x1p.tile([P, 4, U], F32R, tag="x1T", name="x1T")
            lnwb = None
            if not (ln1_triv and ln2_triv):
                lnwb = x1p.tile([P, 4, D], F32, tag="lnwb", name="lnwb")
                for k in range(4):
                    src = bass.AP(tensor=lnw.tensor, offset=k * D,
                                  ap=[[0, P], [1, D]])
                    nc.gpsimd.dma_start(lnwb[:, k, :], src)


            # ======================= PHASE 3: X1 = LN1 ======================
            x1w = pool("x1w", 1, side="right")
            lns = pool("lns", 8, side="right")
            psc = pool("psc", 3, space="PSUM")
            pst = pool("pst", 3, space="PSUM")

            wo_t = x1w.tile([P, 4, D], F32R, tag="wo", name="wot")
            for c in range(4):
                nc.sync.dma_start(wo_t[:, c, :], wo[c])
            xpbo_t = x1w.tile([P, 8, D], F32, tag="xpbo", name="xpbot")
            for t in range(8):
                nc.sync.dma_start(xpbo_t[:, t, :], xpbo[t])

            for t in range(0 if "x1" in SKIP else 8):
                ps = psc.tile([P, D], F32, tag="psx1", name="psx1")
                for c in range(4):
                    nc.tensor.matmul(
                        ps,
                        attnT[:, c, t * P:(t + 1) * P],
                        wo_t[:, c, :],
                        start=(c == 0), stop=(c == 3),
                    )
                o1 = lns.tile([P, D], F32, tag="o1", name="o1")
                nc.vector.tensor_tensor(o1, ps, xpbo_t[:, t, :], ALU.add)
                ps = o1
                stats = lns.tile([P, 6], F32, tag="st", name="st")
                nc.vector.bn_stats(stats, ps)
                mv = lns.tile([P, 2], F32, tag="mv", name="mv")
                nc.vector.bn_aggr(mv, stats)
                rstd = lns.tile([P, 1], F32, tag="rstd", name="rstd")
                nc.scalar.activation(rstd, mv[:, 1:2], AF.Sqrt, bias=epst)
                nc.vector.reciprocal(rstd, rstd)
                if ln1_triv and not apply_mask:
                    nc.vector.tensor_scalar(
                        x1_t[:, t, :], ps, mv[:, 0:1], rstd,
                        ALU.subtract, ALU.mult,
                    )
                elif ln1_triv:
                    xh = lns.tile([P, D], F32, tag="xh", name="xh")
                    nc.vector.tensor_scalar(
                        xh, ps, mv[:, 0:1], rstd, ALU.subtract, ALU.mult
                    )
                    nc.vector.tensor_scalar_mul(
                        x1_t[:, t, :], xh, valid_t[:, t:t + 1],
                    )
                else:
                    xh = lns.tile([P, D], F32, tag="xh", name="xh")
                    nc.vector.tensor_scalar(
                        xh, ps, mv[:, 0:1], rstd, ALU.subtract, ALU.mult
                    )
                    xg = lns.tile([P, D], F32, tag="xg", name="xg")
                    nc.vector.tensor_tensor(xg, xh, lnwb[:, 0, :], ALU.mult)
                    if apply_mask:
                        nc.vector.tensor_tensor(xg, xg, lnwb[:, 1, :], ALU.add)
                        nc.vector.tensor_scalar_mul(
                            x1_t[:, t, :], xg, valid_t[:, t:t + 1],
                        )
                    else:
                        nc.vector.tensor_tensor(
                            x1_t[:, t, :], xg, lnwb[:, 1, :], ALU.add
                        )
                for c in range(4):
                    pt = pst.tile([P, P], F32, tag="pstr", name="pstr")
                    nc.tensor.transpose(
                        pt, x1_t[:, t, c * P:(c + 1) * P], identf
                    )
                    nc.scalar.activation(
                        x1T_t[:, c, t * P:(t + 1) * P], pt, AF.Copy
                    )

            close("pst", "psc", "lns", "x1w")
            close("mid")

            # ========================= PHASE 4: FFN =========================
            ffnw = pool("ffnw", 1)  # left stack: consts, x1p, ffnw
            hidp = pool("hidp", 1, side="right")
            w2s = pool("w2s", 6, side="right")
            lns2 = pool("lns2", 8, side="right")
            outp = pool("outp", 2, side="right")
            psf = pool("psf", 4, space="PSUM")

            w1_t = ffnw.tile([P, 4, DFF], F32R, tag="w1", name="w1t")
            for c in range(4):
                nc.sync.dma_start(w1_t[:, c, :], w1[c])

            ublk = UBLK if (ln1_triv and ln2_triv and not apply_mask) else 256
            nblk = 0 if "ffn" in SKIP else U // ublk
            for ub in range(nblk):
                hid = hidp.tile([P, 32, ublk], F32R, tag="hid", name="hid")
                for t in range(32):
                    ps = psf.tile([P, ublk], F32, tag="psh", name="psh")
                    for c in range(4):
                        nc.tensor.matmul(
                            ps,
                            w1_t[:, c, t * P:(t + 1) * P],
                            x1T_t[:, c, ub * ublk:(ub + 1) * ublk],
                            start=(c == 0), stop=(c == 3),
                        )
                    nc.scalar.activation(
                        hid[:, t, :], ps, AF.Relu, bias=bf1_t[:, t:t + 1],
                    )
                nv = ublk // P
                psos = [
                    psf.tile([P, D], F32, tag="pso", name=f"pso{v}")
                    for v in range(nv)
                ]
                for c in range(33):
                    w2c = w2s.tile([P, D], F32R, tag="w2c", name="w2c")
                    nc.sync.dma_start(w2c, w2[c])
                    for v in range(nv):
                        if c < 32:
                            nc.tensor.matmul(
                                psos[v],
                                hid[:, c, v * P:(v + 1) * P],
                                w2c,
                                start=(c == 0), stop=False,
                                skip_group_check=True,
                            )
                        else:
                            nc.tensor.matmul(
                                psos[v], ones_bf, w2c[0:1, :],
                                start=False, stop=True, skip_group_check=True,
                            )
                for v in range(nv):
                    g = ub * nv + v
                    ps = psos[v]
                    x2p = lns2.tile([P, D], F32, tag="x2p", name="x2p")
                    nc.vector.tensor_tensor(x2p, ps, x1_t[:, g, :], ALU.add)
                    ps = x2p
                    stats = lns2.tile([P, 6], F32, tag="st2", name="st2")
                    nc.vector.bn_stats(stats, ps)
                    mv = lns2.tile([P, 2], F32, tag="mv2", name="mv2")
                    nc.vector.bn_aggr(mv, stats)
                    rstd = lns2.tile([P, 1], F32, tag="rstd2", name="rstd2")
                    nc.scalar.activation(rstd, mv[:, 1:2], AF.Sqrt, bias=epst)
                    nc.vector.reciprocal(rstd, rstd)
                    x2 = outp.tile([P, D], F32, tag="x2", name="x2")
                    if ln2_triv and not apply_mask:
                        nc.vector.tensor_scalar(
                            x2, ps, mv[:, 0:1], rstd, ALU.subtract, ALU.mult
                        )
                        nc.sync.dma_start(out[g], x2)
                    elif ln2_triv:
                        xh = lns2.tile([P, D], F32, tag="xh2", name="xh2")
                        nc.vector.tensor_scalar(
                            xh, ps, mv[:, 0:1], rstd, ALU.subtract, ALU.mult
                        )
                        nc.vector.tensor_scalar_mul(x2, xh, valid_t[:, g:g + 1])
                        nc.sync.dma_start(out[g], x2)
                    else:
                        xh = lns2.tile([P, D], F32, tag="xh2", name="xh2")
                        nc.vector.tensor_scalar(
                            xh, ps, mv[:, 0:1], rstd, ALU.subtract, ALU.mult
                        )
                        xg = lns2.tile([P, D], F32, tag="xg2", name="xg2")
                        nc.vector.tensor_tensor(xg, xh, lnwb[:, 2, :], ALU.mult)
                        if apply_mask:
                            nc.vector.tensor_tensor(xg, xg, lnwb[:, 3, :], ALU.add)
                            nc.vector.tensor_scalar_mul(
                                x2, xg, valid_t[:, g:g + 1]
                            )
                        else:
                            nc.vector.tensor_tensor(x2, xg, lnwb[:, 3, :], ALU.add)
                        nc.sync.dma_start(out[g], x2)

            close("psf", "outp", "lns2", "w2s", "hidp", "ffnw", "x1p", "consts")
        finally:
            for n in list(open_cms):
                try:
                    open_cms.pop(n).__exit__(None, None, None)
                except Exception:
                    pass

    nc.compile()
    return nc


def _get_program(fast_gates, apply_mask, ncat, ln1_triv=False, ln2_triv=False):
    key = (fast_gates, apply_mask, ncat, ln1_triv, ln2_triv)
    if key not in _prog_cache:
        _prog_cache[key] = _build_program(fast_gates, apply_mask, ncat,
                                          ln1_triv, ln2_triv)
    return _prog_cache[key]


def kernel(**inputs):
    X = np.ascontiguousarray(np.asarray(inputs["X"], dtype=np.float32))
    mask = np.asarray(inputs["mask_u"]).astype(bool)
    spk = np.asarray(inputs["speakers"]).astype(np.int64)
    Wq = np.asarray(inputs["Wq"], np.float32); bq = np.asarray(inputs["bq"], np.float32)
    Wk = np.asarray(inputs["Wk"], np.float32); bk = np.asarray(inputs["bk"], np.float32)
    Wv = np.asarray(inputs["Wv"], np.float32); bv = np.asarray(inputs["bv"], np.float32)
    Wo = np.asarray(inputs["Wo"], np.float32); bo = np.asarray(inputs["bo"], np.float32)
    relb = np.asarray(inputs["rel_bias"], np.float32)
    gate = np.asarray(inputs["speaker_gate"], np.float32)
    sims = np.asarray(inputs["sim_scale"], np.float32)
    g1 = np.asarray(inputs["g1"], np.float32); beta1 = np.asarray(inputs["beta1"], np.float32)
    g2 = np.asarray(inputs["g2"], np.float32); beta2 = np.asarray(inputs["beta2"], np.float32)
    W1 = np.asarray(inputs["W1"], np.float32); bf1 = np.asarray(inputs["bf1"], np.float32)
    W2 = np.asarray(inputs["W2"], np.float32); bf2 = np.asarray(inputs["bf2"], np.float32)

    ncat = int(max(9, spk.max() + 1))
    fast_gates = bool(np.all(gate == gate[0]) and np.all(sims == sims[0]))
    apply_mask = not bool(mask.all())

    ln1_triv = bool(np.all(g1 == 1.0) and np.all(beta1 == 0.0))
    ln2_triv = bool(np.all(g2 == 1.0) and np.all(beta2 == 0.0))
    nc = _get_program(fast_gates, apply_mask, ncat, ln1_triv, ln2_triv)

    # ---- shared (weight) arrays ----
    scale = 1.0 / math.sqrt(DH)
    wq_a = np.ascontiguousarray((Wq * scale).reshape(4, P, D))
    wk_a = np.ascontiguousarray(Wk.reshape(4, P, D))
    wv_a = np.concatenate([Wv.reshape(4, P, D), np.zeros((1, P, D), np.float32)], 0)
    wv_a[4, 0, :] = bv
    wv_a = np.ascontiguousarray(wv_a)
    wo_a = np.ascontiguousarray(Wo.reshape(4, P, D))
    w1_a = np.ascontiguousarray(W1.reshape(4, P, DFF))
    w2_a = np.concatenate([W2.reshape(32, P, D), np.zeros((1, P, D), np.float32)], 0)
    w2_a[32, 0, :] = bf2
    w2_a = np.ascontiguousarray(w2_a)
    bf1p_a = np.ascontiguousarray(bf1.reshape(32, P).T)
    qkb_a = np.zeros((P, 8), np.float32)
    qkb_a[:, 0:4] = (bq * scale).reshape(4, P).T
    qkb_a[:, 4:8] = bk.reshape(4, P).T
    lnw_a = np.ascontiguousarray(np.stack([g1, beta1, g2, beta2]))

    # banded rel bias: rb[a, h, o, c] = relb[h, min(|(o-1)*128+c-a|,128)] - relb[h,128]
    a_i = np.arange(P)[:, None]
    c_i = np.arange(P)[None, :]
    rb_hoc = np.zeros((H, 3, P, P), np.float32)
    for o in range(3):
        dist = np.minimum(np.abs((o - 1) * P + c_i - a_i), REL_MAX)
        rb_hoc[:, o] = relb[:, dist] - relb[:, REL_MAX][:, None, None]
    rbd_a = np.ascontiguousarray(rb_hoc.transpose(2, 0, 1, 3))  # [a, h, o, c]

    # denominator-broadcast expander: r = (h - 2c)*2 + j
    expd_a = np.zeros((4, 2, P), np.float32)
    for j in range(2):
        expd_a[j, j, 0:64] = 1.0
        expd_a[2 + j, j, 64:P] = 1.0
    expd_a = np.ascontiguousarray(expd_a.reshape(4, 2 * P))

    ident_a = np.eye(P, dtype=np.float32)
    uvec4_a = np.ascontiguousarray(np.eye(4, dtype=np.float32).reshape(1, 16))

    shared = dict(wq=wq_a, wk=wk_a, wv=wv_a, wo=wo_a, w1=w1_a, w2=w2_a,
                  bf1p=bf1p_a, qkb=qkb_a, lnw=lnw_a, rbd=rbd_a, expd=expd_a,
                  identd=ident_a, identfd=ident_a, uvec4=uvec4_a,
                  ones_pe=np.ones((1, P), np.float32),
                  ones_v=np.ones((P, 64), np.float32))
    if not fast_gates:
        shared["sidents"] = np.ascontiguousarray(sims[:, None, None] * ident_a[None])
        shared["gidents"] = np.ascontiguousarray(-gate[:, None, None] * ident_a[None])

    in_maps = []
    for b in range(B):
        Xb = X[b]
        validf = mask[b].astype(np.float32)
        norm = np.linalg.norm(Xb, axis=-1)
        rn = (1.0 / np.maximum(norm, 1e-6)) * validf
        Pmat = np.zeros((U, ncat), np.float32)
        Pmat[np.arange(U), np.clip(spk[b], 0, ncat - 1)] = 1.0
        ptb_a = np.ascontiguousarray(Pmat.T)
        pta_a = np.ascontiguousarray((-gate)[:, None, None] * ptb_a[None])
        m = dict(
            xt=np.ascontiguousarray(Xb.T).reshape(4, P, U),
            xpbo=np.ascontiguousarray((Xb + bo).reshape(8, P, D)),
            rns_a=np.ascontiguousarray((sims[0] * rn)[None, :]),
            rns_b=np.ascontiguousarray(rn[None, :]),
            pta=pta_a,
            ptb=ptb_a,
            validd=np.ascontiguousarray(validf.reshape(8, P).T),
            **shared,
        )
        in_maps.append(m)

    res = run_bass_kernel_spmd(nc, in_maps, core_ids=list(range(NCORES)))
    outs = [r["out"].reshape(U, D) for r in res.results]
    return np.stack(outs).astype(np.float32)



# revision 36
# speedup vs baseline: 1.4424x; 1.4424x over previous
"""Trainium2 Bass kernel for nn_BiasedMHABlock (biased MHA + FFN transformer block).

Sharding: batch B=8 -> one batch per NeuronCore (SPMD, no collectives).

All heavy matmuls run as fp8e4 (e4m3) with MatmulPerfMode.DoubleRow: each
matmul instruction contracts two 128-row k-tiles (lhsT/rhs shaped [K,2,*])
at 0.5 PE cycles per output column -- 4x the fp32r FLOP rate.

Scale bookkeeping (all powers of two, exact):
  weights Wq(/8)/Wk/Wv/Wo/W1/W2 stored as 32x in fp8; X stored unscaled fp8.
  q,k true scale (activation scale 2^-5 on the 32x psum); vt = 32*v.
  scores psum is true scale: the per-head DoubleRow pairs (K_h | I/32) x
  (Q | 32*CB) and (I/32 | I/32) x (32*relband | 0) add the cosine/speaker
  bias CB and the banded relative-position bias inside the score matmul.
  et = exp(scores) in fp8; attn@V pairs w-tiles: (V 2-tiles | .) x (et pairs).
  attnT = 32*attn_out (fp8) -> Wo psum = 1024*(attn@Wo); xpbo = 1024*(X+bo);
  LN1 emits x1_t = 2048*x1 (f32, via rstd scale trick) and x1T = 32*x1 (fp8).
  W1 psum = 1024*z; hid = 64*relu(z) (fp8); W2 psum = 2048*ffn; residual add
  is scale-matched; LN2 normalization cancels the 2048 exactly.
Softmax runs over the partition axis without max-subtraction (scores are
O(1)); the denominator comes from an appended ones-column of V and is
divided out post-hoc (free columns of the same DoubleRow matmuls).
"""
import sys
import math

import os
for _p in ("/opt/trn_rl_repo", "/root/.axon_site/_ro/trn_rl_repo"):
    if os.path.isdir(_p) and _p not in sys.path:
        sys.path.insert(0, _p)

import numpy as np
import ml_dtypes

import concourse.bass as bass
import concourse.tile as tile
from concourse import bacc, mybir
from concourse.bass_utils import run_bass_kernel_spmd

F32 = mybir.dt.float32
F32R = mybir.dt.float32r
BF16 = mybir.dt.bfloat16
F8 = mybir.dt.float8e4
F8NP = mybir.dt.np(F8)
AF = mybir.ActivationFunctionType
ALU = mybir.AluOpType
PM = mybir.MatmulPerfMode

B, U, D, H, DH, DFF = 8, 1024, 512, 8, 64, 4096
REL_MAX = 128
P = 128
NCORES = 8
LN_EPS = 1e-5
NCAT = 16  # padded speaker-category partitions

_prog_cache = {}


def _drp(a0, a1):
    """DoubleRow pair AP from two same-shape 2-dim slices of one tile."""
    s = a1.offset - a0.offset
    return bass.AP(tensor=a0.tensor, offset=a0.offset,
                   ap=[list(a0.ap[0]), [s, 2], list(a0.ap[-1])])


def _build_program(fast_gates, apply_mask, ln1_triv, ln2_triv,
                   bf1_nz, bf2_nz, bv_nz):
    nc = bacc.Bacc("TRN2", target_bir_lowering=False, debug=False)

    def din(name, shape, dt=F8):
        return nc.dram_tensor(name, list(shape), dt, kind="ExternalInput").ap()

    xt8 = din("xt8", [4, P, U])
    xpbo = din("xpbo", [8, P, D], F32)
    rns_b = din("rns_b", [1, U], F32)
    if fast_gates:
        rns_a = din("rns_a", [1, U], F32)
    pta2 = din("pta2", [NCAT, 2, U])
    ptb2 = din("ptb2", [NCAT, 2, U])
    wq8 = din("wq8", [4, P, D])
    wk8 = din("wk8", [4, P, D])
    wv8 = din("wv8", [4, P, D])
    wo8 = din("wo8", [4, P, D])
    w18 = din("w18", [2, 4, P, DFF])   # fp8 residual split: [a;b] halves
    w28 = din("w28", [2, 32, P, D])
    qkb = din("qkb", [P, 8], F32)
    bf1p = din("bf1p", [P, 32], F32)
    rbz8 = din("rbz8", [P, H, 6, P])
    i32d = din("i32d", [P, P])            # I * 2^-5
    identfd = din("identfd", [P, P], F32)
    expd = din("expd", [4, 2 * P], F32R)
    uvec4 = din("uvec4", [1, 16], F32R)
    ones_v8 = din("ones_v8", [P, 8, 8, 16])
    zpad = din("zpad", [64, 4, U])        # fp8 zeros for kti pads
    lnw = din("lnw", [4, D], F32)
    validd = din("validd", [P, 8], F32)
    if bv_nz:
        bvp = din("bvp", [1, D], F32)     # 32*bv
    if bf2_nz:
        bf2p = din("bf2p", [1, D], F32)   # 2048*bf2
    if not fast_gates:
        sid8 = din("sid8", [P, H, P])     # sims[h] * I
        gid8 = din("gid8", [P, H, 2, P])  # [gate[h]*I ; 0]

    out = nc.dram_tensor("out", [8, P, D], F32, kind="ExternalOutput").ap()

    # qcb free-slot layout: 0:4 q packed, 4:12 cb (or sim), 12:20 eq(non-fast)
    NQ = 12 if fast_gates else 20
    open_cms = {}

    with tile.TileContext(nc) as tc, nc.allow_low_precision(reason="fp8 kernel"):
        def pool(name, bufs, space="SBUF", side="left"):
            cm = tc.tile_pool(name=name, bufs=bufs, space=space, side=side)
            p = cm.__enter__()
            open_cms[name] = cm
            return p

        def close(*names):
            for n in names:
                open_cms.pop(n).__exit__(None, None, None)

        try:
            # ---------------- constants (left, whole-kernel) ----------------
            consts = pool("consts", 1)
            identf = consts.tile([P, P], F32)
            idd = consts.tile([P, 2, P], F8)
            qkb_t = consts.tile([P, 8], F32)
            bf1_t = consts.tile([P, 32], F32)
            valid_t = consts.tile([P, 8], F32)
            eps1 = consts.tile([P, 1], F32)
            eps2 = consts.tile([P, 1], F32)

            # w2/xpbo preload pool: opened before attn_in (LIFO), DMAs issued
            # at phase-2 start so they overlap with attention compute
            w2p = pool("w2p", 1)
            w2_t = w2p.tile([P, 2, 32, D], F8, tag="w2", name="w2t")
            xpbo_t = w2p.tile([P, 8, D], F32, tag="xpbo", name="xpbot")

            # ------------- long-lived attention inputs (left) ----------------
            attn_in = pool("attn_in", 1)
            kti = attn_in.tile([P, H, 1152], F8, tag="kti", name="kti")
            qcb = attn_in.tile([P, NQ, U], F8, tag="qcb", name="qcb")
            vt_t = attn_in.tile([P, 8, 640], F8, tag="vt", name="vt")
            rbz = attn_in.tile([P, H, 6, P], F8, tag="rbz", name="rbz")
            if not fast_gates:
                gid2 = attn_in.tile([P, H, 2, P], F8, tag="gid", name="gid")
                nc.sync.dma_start(gid2, gid8)

            # ======================= PHASE 1: prep ==========================
            pre = pool("pre", 1, side="right")
            prew = pool("prew", 3, side="right")
            ps1 = pool("ps1", 2, space="PSUM")

            xt_t = pre.tile([P, 4, U], F8, tag="xt", name="xtt")
            wq_t = prew.tile([P, 4, D], F8, tag="wx", name="wqt")
            wk_t = prew.tile([P, 4, D], F8, tag="wx", name="wkt")
            for c in range(4):
                nc.sync.dma_start(xt_t[:, c, :], xt8[c])
                nc.sync.dma_start(wq_t[:, c, :], wq8[c])
                nc.sync.dma_start(wk_t[:, c, :], wk8[c])
            nc.sync.dma_start(qkb_t, qkb)
            nc.sync.dma_start(identf, identfd)
            nc.sync.dma_start(
                idd, bass.AP(tensor=i32d.tensor, offset=0,
                             ap=[[P, P], [0, 2], [1, P]]))
            nc.vector.memset(eps1, LN_EPS / 4.0)
            nc.vector.memset(eps2, LN_EPS * float(2 ** 22))
            if bf1_nz:
                nc.sync.dma_start(bf1_t, bf1p)
            if apply_mask:
                nc.sync.dma_start(valid_t, validd)
            if not (ln1_triv and ln2_triv):
                lnwb = consts.tile([P, 4, D], F32)
                for k in range(4):
                    src = bass.AP(tensor=lnw.tensor, offset=k * D,
                                  ap=[[0, P], [1, D]])
                    nc.gpsimd.dma_start(lnwb[:, k, :], src)
            # kti pads: zero the complementary 64-partition halves
            _lo = kti[64:128, 0:1, 0:U]   # even-head slots, partitions 64..127
            nc.sync.dma_start(
                bass.AP(tensor=_lo.tensor, offset=_lo.offset,
                        ap=[list(_lo.ap[0]), [2 * 1152, 4], [1, U]]),
                zpad)
            _hi = kti[0:64, 1:2, 0:U]     # odd-head slots, partitions 0..63
            nc.sync.dma_start(
                bass.AP(tensor=_hi.tensor, offset=_hi.offset,
                        ap=[list(_hi.ap[0]), [2 * 1152, 4], [1, U]]),
                zpad)
            # kti ident region (per-head I/32, or sims[h]*I when not fast)
            if fast_gates:
                nc.sync.dma_start(
                    kti[:, :, 1024:1152],
                    bass.AP(tensor=i32d.tensor, offset=0,
                            ap=[[P, P], [0, H], [1, P]]))
            else:
                nc.sync.dma_start(kti[:, :, 1024:1152], sid8)

            rnsb_b = pre.tile([P, U], F32, tag="rnsb", name="rnsb")
            nc.gpsimd.dma_start(
                rnsb_b, bass.AP(tensor=rns_b.tensor, offset=0,
                                ap=[[0, P], [1, U]]))
            if fast_gates:
                rnsa_b = pre.tile([P, U], F32, tag="rnsa", name="rnsa")
                nc.gpsimd.dma_start(
                    rnsa_b, bass.AP(tensor=rns_a.tensor, offset=0,
                                    ap=[[0, P], [1, U]]))

            # Q (packed into qcb[:,0:4]) and K (padded per head in kti)
            for t in range(4):
                psq = ps1.tile([P, 2, D], F32, tag="psbig", name="psq")
                psk = ps1.tile([P, 2, D], F32, tag="psbig", name="psk")
                for j in range(2):
                    for cp in range(2):
                        nc.tensor.matmul(
                            psq[:, j, :],
                            wq_t[:, 2 * cp:2 * cp + 2, t * P:(t + 1) * P],
                            xt_t[:, 2 * cp:2 * cp + 2, j * D:(j + 1) * D],
                            start=(cp == 0), stop=(cp == 1),
                            perf_mode=PM.DoubleRow,
                        )
                        nc.tensor.matmul(
                            psk[:, j, :],
                            wk_t[:, 2 * cp:2 * cp + 2, t * P:(t + 1) * P],
                            xt_t[:, 2 * cp:2 * cp + 2, j * D:(j + 1) * D],
                            start=(cp == 0), stop=(cp == 1),
                            perf_mode=PM.DoubleRow,
                        )
                nc.scalar.activation(
                    qcb[:, t, :], psq[:, 0:2, :], AF.Identity,
                    bias=qkb_t[:, t:t + 1], scale=2.0 ** -5,
                )
                nc.scalar.activation(
                    kti[0:64, 2 * t, 0:U], psk[0:64, 0:2, :], AF.Identity,
                    bias=qkb_t[0:64, 4 + t:5 + t], scale=2.0 ** -5,
                )
                nc.scalar.activation(
                    kti[64:128, 2 * t + 1, 0:U], psk[64:128, 0:2, :],
                    AF.Identity,
                    bias=qkb_t[64:128, 4 + t:5 + t], scale=2.0 ** -5,
                )

            # V (interleaved (h dh) layout + ones cols); vt = 32*v
            wv_t = prew.tile([P, 4, D], F8, tag="wx", name="wvt")
            for c in range(4):
                nc.sync.dma_start(wv_t[:, c, :], wv8[c])
            nc.sync.dma_start(
                vt_t.rearrange("p t (h c) -> p t h c", c=80)[:, :, :, 64:80],
                ones_v8,
            )
            if bv_nz:
                bvp_b = pre.tile([P, D], F32, tag="bvp", name="bvp")
                nc.gpsimd.dma_start(
                    bvp_b, bass.AP(tensor=bvp.tensor, offset=0,
                                   ap=[[0, P], [1, D]]))
            for t in range(8):
                psv = ps1.tile([P, D], F32, tag="psv", name="psv")
                for cp in range(2):
                    nc.tensor.matmul(
                        psv,
                        xt_t[:, 2 * cp:2 * cp + 2, t * P:(t + 1) * P],
                        wv_t[:, 2 * cp:2 * cp + 2, :],
                        start=(cp == 0), stop=(cp == 1),
                        perf_mode=PM.DoubleRow,
                    )
                vdst = vt_t[:, t, :].rearrange(
                    "p (h c) -> p h c", c=80)[:, :, 0:64]
                vsrc = psv.rearrange("p (h dh) -> p h dh", h=H)
                if bv_nz:
                    nc.vector.tensor_tensor(
                        vdst, vsrc,
                        bvp_b.rearrange("p (h dh) -> p h dh", h=H), ALU.add)
                else:
                    nc.vector.tensor_copy(vdst, vsrc)
                if apply_mask:
                    nc.vector.tensor_scalar_mul(
                        vt_t[:, t, :], vt_t[:, t, :], valid_t[:, t:t + 1])

            # Xn tiles (16x scaled) and CB = 32*(sim_scale*XnXn^T - gate*SS^T)
            xnb_t = pre.tile([P, 4, U], F8, tag="xnb", name="xnb")
            for c in range(4):
                nc.gpsimd.tensor_tensor(
                    xnb_t[:, c, :], xt_t[:, c, :], rnsb_b, ALU.mult)
            if fast_gates:
                xna_t = pre.tile([P, 4, U], F8, tag="xna", name="xna")
                for c in range(4):
                    nc.gpsimd.tensor_tensor(
                        xna_t[:, c, :], xt_t[:, c, :], rnsa_b, ALU.mult)
            else:
                xna_t = xnb_t

            pta_t = pre.tile([NCAT, 2, U], F8, tag="pta", name="ptat")
            ptb_t = pre.tile([NCAT, 2, U], F8, tag="ptb", name="ptbt")
            nc.sync.dma_start(ptb_t, ptb2)
            if fast_gates:
                nc.sync.dma_start(pta_t, pta2)

            for i in range(8):
                pscb = ps1.tile([P, 2, D], F32, tag="psbig", name="pscb")
                for j in range(2):
                    for cp in range(2):
                        nc.tensor.matmul(
                            pscb[:, j, :],
                            xna_t[:, 2 * cp:2 * cp + 2, i * P:(i + 1) * P],
                            xnb_t[:, 2 * cp:2 * cp + 2, j * D:(j + 1) * D],
                            start=(cp == 0),
                            stop=(cp == 1 and not fast_gates),
                            perf_mode=PM.DoubleRow,
                        )
                    if fast_gates:
                        nc.tensor.matmul(
                            pscb[:, j, :],
                            pta_t[:, 0:2, i * P:(i + 1) * P],
                            ptb_t[:, 0:2, j * D:(j + 1) * D],
                            start=False, stop=True,
                            perf_mode=PM.DoubleRow,
                        )
                # fast: psum = 256*CB -> 32*CB ; nonfast: 256*sim -> sim
                nc.vector.tensor_scalar_mul(
                    qcb[:, 4 + i, :], pscb[:, 0:2, :],
                    (2.0 ** -3 if fast_gates else 2.0 ** -8),
                )
                if not fast_gates:
                    # eq indicator: 256*eq -> eq in qcb[:, 12+i]
                    pseq = ps1.tile([P, 2, D], F32, tag="psbig", name="pseq")
                    for j in range(2):
                        nc.tensor.matmul(
                            pseq[:, j, :],
                            ptb_t[:, 0:1, i * P:(i + 1) * P],
                            ptb_t[:, 0:1, j * D:(j + 1) * D],
                            start=True, stop=True,
                        )
                    nc.vector.tensor_scalar_mul(
                        qcb[:, 12 + i, :], pseq[:, 0:2, :], 2.0 ** -8)

            close("ps1", "prew", "pre")

            nc.sync.dma_start(rbz, rbz8)
            # preload FFN W2 + xpbo now: overlaps with attention compute
            for c in range(32):
                nc.sync.dma_start(w2_t[:, 0, c, :], w28[0, c])
                nc.sync.dma_start(w2_t[:, 1, c, :], w28[1, c])
            for t in range(8):
                nc.sync.dma_start(xpbo_t[:, t, :], xpbo[t])

            # ====================== PHASE 2: attention ======================
            mid = pool("mid", 1, side="right")
            attnT = mid.tile([P, 4, U], F8, tag="attnT", name="attnT")
            den_sb = mid.tile([1, 16, D], F32R, tag="densb", name="densb")
            expd_t = mid.tile([4, 2 * P], F32R, tag="expd", name="expdt")
            nc.sync.dma_start(expd_t, expd)
            uvec_t = mid.tile([1, 16], F32R, tag="uvec", name="uvect")
            nc.sync.dma_start(uvec_t, uvec4)

            epool = pool("epool", 2, side="right")
            arp = pool("arp", 2, side="right")
            dnp = pool("dnp", 4, side="right")
            pss = pool("pss", 2, space="PSUM")
            psa = pool("psa", 4, space="PSUM")

            rb_base = rbz[:, 0, 0, :]

            def rb_pair(h, o0, W):
                off0 = rb_base.offset + (h * 6 + o0) * P
                off1 = rb_base.offset + (h * 6 + 3) * P
                return bass.AP(tensor=rb_base.tensor, offset=off0,
                               ap=[list(rb_base.ap[0]), [off1 - off0, 2],
                                   [1, W]])

            for h in range(H):
                po = (h % 2) * 64
                ch = h // 2
                patts = [
                    psa.tile([65, D], F32, tag="psatt", name=f"psatt_{h}_{j}")
                    for j in range(2)
                ]
                if h % 2 == 0:
                    attnR = arp.tile([P, U], BF16, tag="attnR", name="attnR")
                for ip in range(4):
                    et = epool.tile([P, 2, U], F8, tag="et", name="et")
                    for ii in range(2):
                        i = 2 * ip + ii
                        ps = pss.tile([P, 2, D], F32, tag="pssc", name="pssc")
                        for j in range(2):
                            lhs1 = _drp(kti[:, h, i * P:(i + 1) * P],
                                        kti[:, h, 1024:1152])
                            rhs1 = _drp(qcb[:, ch, j * D:(j + 1) * D],
                                        qcb[:, 4 + i, j * D:(j + 1) * D])
                            lo_b = max(i - 1, 0)
                            hi_b = min(i + 1, 7)
                            run_lo = max(lo_b * P, j * D)
                            run_hi = min((hi_b + 1) * P, (j + 1) * D)
                            has_rel = run_hi > run_lo
                            if not fast_gates:
                                rhs_eq = bass.AP(
                                    tensor=qcb.tensor,
                                    offset=qcb[:, 12 + i,
                                               j * D:(j + 1) * D].offset,
                                    ap=[list(qcb[:, 12 + i, 0:1].ap[0]),
                                        [0, 2], [1, D]])
                            nc.tensor.matmul(
                                ps[:, j, :], lhs1, rhs1,
                                start=True,
                                stop=not (has_rel or not fast_gates),
                                perf_mode=PM.DoubleRow,
                                skip_group_check=True,
                            )
                            if not fast_gates:
                                nc.tensor.matmul(
                                    ps[:, j, :], gid2[:, h, 0:2, :], rhs_eq,
                                    start=False, stop=not has_rel,
                                    perf_mode=PM.DoubleRow,
                                    skip_group_check=True,
                                )
                            if has_rel:
                                o0 = (run_lo // P) - (i - 1)
                                W = run_hi - run_lo
                                nc.tensor.matmul(
                                    ps[:, j, run_lo - j * D:run_hi - j * D],
                                    idd[:, 0:2, :], rb_pair(h, o0, W),
                                    start=False, stop=True,
                                    perf_mode=PM.DoubleRow,
                                    skip_group_check=True,
                                )
                        nc.scalar.activation(
                            et[:, ii, :], ps[:, 0:2, :], AF.Exp)
                    for j in range(2):
                        nc.tensor.matmul(
                            patts[j],
                            vt_t[:, 2 * ip:2 * ip + 2, h * 80:h * 80 + 65],
                            et[:, 0:2, j * D:(j + 1) * D],
                            start=(ip == 0), stop=(ip == 3),
                            perf_mode=PM.DoubleRow,
                        )
                for j in range(2):
                    idx = h * 2 + j
                    nc.vector.tensor_copy(
                        den_sb[0:1, idx, :], patts[j][64:65, :])
                    nc.vector.tensor_copy(
                        attnR[po:po + 64, j * D:(j + 1) * D],
                        patts[j][0:64, :])
                if h % 2 == 1:
                    c4 = 4 * ch
                    psg = psa.tile([4, D], F32, tag="psatt", name=f"psg_{ch}")
                    for r in range(4):
                        nc.tensor.matmul(
                            psg,
                            uvec_t[0:1, r * 4:(r + 1) * 4],
                            den_sb[0:1, c4 + r, :],
                            start=(r == 0), stop=(r == 3),
                        )
                    rden4 = dnp.tile([4, D], F32R, tag="rden4", name="rden4")
                    nc.vector.reciprocal(rden4, psg)
                    for j in range(2):
                        psn = psa.tile([P, D], F32, tag="psatt",
                                       name=f"psn_{ch}_{j}")
                        nc.tensor.matmul(
                            psn, expd_t[:, j * P:(j + 1) * P], rden4,
                            start=True, stop=True,
                        )
                        nc.vector.tensor_tensor(
                            attnT[:, ch, j * D:(j + 1) * D],
                            attnR[:, j * D:(j + 1) * D],
                            psn, ALU.mult,
                        )

            close("psa", "pss", "dnp", "arp", "epool")
            close("attn_in")

            # ---------- x1 pool opens early on the left (outlives mid) -------
            x1p = pool("x1p", 1)
            x1_t = x1p.tile([P, 8, D], F32, tag="x1", name="x1")
            x1T_t = x1p.tile([P, 4, U], F8, tag="x1T", name="x1T")

            # ================== PHASE 3: X1 = LN1(X+bo+attn@Wo) =============
            x1w = pool("x1w", 1, side="right")
            lns = pool("lns", 8, side="right")
            psc = pool("psc", 3, space="PSUM")
            pst = pool("pst", 3, space="PSUM")

            wo_t = x1w.tile([P, 4, D], F8, tag="wo", name="wot")
            for c in range(4):
                nc.sync.dma_start(wo_t[:, c, :], wo8[c])

            for t in range(8):
                ps = psc.tile([P, D], F32, tag="psx1", name="psx1")
                for cp in range(2):
                    nc.tensor.matmul(
                        ps,
                        attnT[:, 2 * cp:2 * cp + 2, t * P:(t + 1) * P],
                        wo_t[:, 2 * cp:2 * cp + 2, :],
                        start=(cp == 0), stop=(cp == 1),
                        perf_mode=PM.DoubleRow,
                    )
                o1 = lns.tile([P, D], F32, tag="o1", name="o1")
                nc.vector.tensor_tensor(o1, ps, xpbo_t[:, t, :], ALU.add)
                stats = lns.tile([P, 6], F32, tag="st", name="st")
                nc.vector.bn_stats(stats, o1)
                mv = lns.tile([P, 2], F32, tag="mv", name="mv")
                nc.vector.bn_aggr(mv, stats)
                rstd = lns.tile([P, 1], F32, tag="rstd", name="rstd")
                nc.scalar.activation(rstd, mv[:, 1:2], AF.Sqrt,
                                     bias=eps1, scale=2.0 ** -22)
                nc.vector.reciprocal(rstd, rstd)
                # x1_t = 2048*x1
                if ln1_triv and not apply_mask:
                    nc.vector.tensor_scalar(
                        x1_t[:, t, :], o1, mv[:, 0:1], rstd,
                        ALU.subtract, ALU.mult,
                    )
                else:
                    xh = lns.tile([P, D], F32, tag="xh", name="xh")
                    nc.vector.tensor_scalar(
                        xh, o1, mv[:, 0:1], rstd, ALU.subtract, ALU.mult)
                    if not ln1_triv:
                        xg = lns.tile([P, D], F32, tag="xg", name="xg")
                        nc.vector.tensor_tensor(xg, xh, lnwb[:, 0, :],
                                                ALU.mult)
                        nc.vector.tensor_tensor(xg, xg, lnwb[:, 1, :],
                                                ALU.add)
                        xh = xg
                    if apply_mask:
                        nc.vector.tensor_scalar_mul(
                            x1_t[:, t, :], xh, valid_t[:, t:t + 1])
                    else:
                        nc.vector.tensor_copy(x1_t[:, t, :], xh)
                for c in range(4):
                    pt = pst.tile([P, P], F32, tag="pstr", name="pstr")
                    nc.tensor.transpose(
                        pt, x1_t[:, t, c * P:(c + 1) * P], identf)
                    nc.vector.tensor_scalar_mul(
                        x1T_t[:, c, t * P:(t + 1) * P], pt, 2.0 ** -6)

            close("pst", "psc", "lns", "x1w")
            close("mid")

            # ========================= PHASE 4: FFN =========================
            ffnw = pool("ffnw", 1)
            hidp = pool("hidp", 1, side="right")
            lns2 = pool("lns2", 8, side="right")
            outp = pool("outp", 2, side="right")
            psf = pool("psf", 4, space="PSUM")

            w1_t = ffnw.tile([P, 2, 4, DFF], F8, tag="w1", name="w1t")
            for hf in range(2):
                for c in range(4):
                    nc.sync.dma_start(w1_t[:, hf, c, :], w18[hf, c])
            if bf2_nz:
                bf2_b = ffnw.tile([P, D], F32, tag="bf2b", name="bf2b")
                nc.gpsimd.dma_start(
                    bf2_b, bass.AP(tensor=bf2p.tensor, offset=0,
                                   ap=[[0, P], [1, D]]))

            ublk = 512
            for ub in range(U // ublk):
                hid = hidp.tile([P, 32, ublk], F8, tag="hid", name="hid")
                for t in range(32):
                    psh = psf.tile([P, ublk], F32, tag="psh", name="psh")
                    for hf in range(2):
                        for cp in range(2):
                            nc.tensor.matmul(
                                psh,
                                w1_t[:, hf, 2 * cp:2 * cp + 2,
                                     t * P:(t + 1) * P],
                                x1T_t[:, 2 * cp:2 * cp + 2,
                                      ub * ublk:(ub + 1) * ublk],
                                start=(hf == 0 and cp == 0),
                                stop=(hf == 1 and cp == 1),
                                perf_mode=PM.DoubleRow,
                            )
                    # hid = 64*relu(z); psum = 1024*z
                    if bf1_nz:
                        nc.scalar.activation(
                            hid[:, t, :], psh, AF.Relu,
                            bias=bf1_t[:, t:t + 1], scale=2.0 ** -10)
                    elif t % 2 == 0:
                        nc.vector.tensor_scalar(
                            hid[:, t, :], psh, 0.0, 2.0 ** -4,
                            ALU.max, ALU.mult)
                    else:
                        nc.scalar.activation(
                            hid[:, t, :], psh, AF.Relu, scale=2.0 ** -4)
                nv = ublk // P
                psos = [
                    psf.tile([P, D], F32, tag="pso", name=f"pso{v}")
                    for v in range(nv)
                ]
                for c2 in range(16):
                    for v in range(nv):
                        for hf in range(2):
                            nc.tensor.matmul(
                                psos[v],
                                hid[:, 2 * c2:2 * c2 + 2, v * P:(v + 1) * P],
                                w2_t[:, hf, 2 * c2:2 * c2 + 2, :],
                                start=(c2 == 0 and hf == 0),
                                stop=(c2 == 15 and hf == 1),
                                perf_mode=PM.DoubleRow,
                                skip_group_check=True,
                            )
                for v in range(nv):
                    g = ub * nv + v
                    x2p = lns2.tile([P, D], F32, tag="x2p", name="x2p")
                    nc.vector.tensor_tensor(x2p, psos[v], x1_t[:, g, :],
                                            ALU.add)
                    if bf2_nz:
                        nc.vector.tensor_tensor(x2p, x2p, bf2_b, ALU.add)
                    stats = lns2.tile([P, 6], F32, tag="st2", name="st2")
                    nc.vector.bn_stats(stats, x2p)
                    mv = lns2.tile([P, 2], F32, tag="mv2", name="mv2")
                    nc.vector.bn_aggr(mv, stats)
                    rstd = lns2.tile([P, 1], F32, tag="rstd2", name="rstd2")
                    nc.scalar.activation(rstd, mv[:, 1:2], AF.Sqrt,
                                         bias=eps2, scale=1.0)
                    nc.vector.reciprocal(rstd, rstd)
                    x2 = outp.tile([P, D], F32, tag="x2", name="x2")
                    if ln2_triv and not apply_mask:
                        nc.vector.tensor_scalar(
                            x2, x2p, mv[:, 0:1], rstd,
                            ALU.subtract, ALU.mult)
                    else:
                        xh = lns2.tile([P, D], F32, tag="xh2", name="xh2")
                        nc.vector.tensor_scalar(
                            xh, x2p, mv[:, 0:1], rstd,
                            ALU.subtract, ALU.mult)
                        if not ln2_triv:
                            xg = lns2.tile([P, D], F32, tag="xg2", name="xg2")
                            nc.vector.tensor_tensor(xg, xh, lnwb[:, 2, :],
                                                    ALU.mult)
                            nc.vector.tensor_tensor(xg, xg, lnwb[:, 3, :],
                                                    ALU.add)
                            xh = xg
                        if apply_mask:
                            nc.vector.tensor_scalar_mul(
                                x2, xh, valid_t[:, g:g + 1])
                        else:
                            nc.vector.tensor_copy(x2, xh)
                    nc.sync.dma_start(out[g], x2)

            close("psf", "outp", "lns2", "hidp", "ffnw", "x1p",
                  "w2p", "consts")
        finally:
            for n in list(open_cms):
                try:
                    open_cms.pop(n).__exit__(None, None, None)
                except Exception:
                    pass

    nc.compile()
    return nc


def _get_program(*key):
    if key not in _prog_cache:
        _prog_cache[key] = _build_program(*key)
    return _prog_cache[key]


def kernel(**inputs):
    X = np.ascontiguousarray(np.asarray(inputs["X"], dtype=np.float32))
    mask = np.asarray(inputs["mask_u"]).astype(bool)
    spk = np.asarray(inputs["speakers"]).astype(np.int64)
    Wq = np.asarray(inputs["Wq"], np.float32); bq = np.asarray(inputs["bq"], np.float32)
    Wk = np.asarray(inputs["Wk"], np.float32); bk = np.asarray(inputs["bk"], np.float32)
    Wv = np.asarray(inputs["Wv"], np.float32); bv = np.asarray(inputs["bv"], np.float32)
    Wo = np.asarray(inputs["Wo"], np.float32); bo = np.asarray(inputs["bo"], np.float32)
    relb = np.asarray(inputs["rel_bias"], np.float32)
    gate = np.asarray(inputs["speaker_gate"], np.float32)
    sims = np.asarray(inputs["sim_scale"], np.float32)
    g1 = np.asarray(inputs["g1"], np.float32); beta1 = np.asarray(inputs["beta1"], np.float32)
    g2 = np.asarray(inputs["g2"], np.float32); beta2 = np.asarray(inputs["beta2"], np.float32)
    W1 = np.asarray(inputs["W1"], np.float32); bf1 = np.asarray(inputs["bf1"], np.float32)
    W2 = np.asarray(inputs["W2"], np.float32); bf2 = np.asarray(inputs["bf2"], np.float32)

    fast_gates = bool(np.all(gate == gate[0]) and np.all(sims == sims[0]))
    apply_mask = not bool(mask.all())
    ln1_triv = bool(np.all(g1 == 1.0) and np.all(beta1 == 0.0))
    ln2_triv = bool(np.all(g2 == 1.0) and np.all(beta2 == 0.0))
    bf1_nz = bool(np.any(bf1 != 0.0))
    bf2_nz = bool(np.any(bf2 != 0.0))
    bv_nz = bool(np.any(bv != 0.0))
    nc = _get_program(fast_gates, apply_mask, ln1_triv, ln2_triv,
                      bf1_nz, bf2_nz, bv_nz)

    f8 = lambda a: np.ascontiguousarray(a).astype(F8NP)
    scale = 1.0 / math.sqrt(DH)

    wq_a = f8((Wq * (scale * 32.0)).reshape(4, P, D))
    wk_a = f8((Wk * 32.0).reshape(4, P, D))
    wv_a = f8((Wv * 32.0).reshape(4, P, D))
    wo_a = f8((Wo * 32.0).reshape(4, P, D))
    def split8(w):
        a = w.astype(F8NP).astype(np.float32)
        b = (w - a).astype(F8NP).astype(np.float32)
        return np.stack([a, b]).astype(F8NP)

    w1_a = np.ascontiguousarray(split8((W1 * 32.0).reshape(4, P, DFF)))
    hid_scale = 1.0 if bf1_nz else 64.0
    w2_a = np.ascontiguousarray(
        split8((W2 * (2048.0 / hid_scale)).reshape(32, P, D)))
    bf1p_a = np.ascontiguousarray(bf1.reshape(32, P).T)
    qkb_a = np.zeros((P, 8), np.float32)
    qkb_a[:, 0:4] = (bq * scale).reshape(4, P).T
    qkb_a[:, 4:8] = bk.reshape(4, P).T
    lnw_a = np.ascontiguousarray(
        np.stack([g1, beta1 * 2048.0, g2, beta2]))

    # banded rel bias (32x): rbz[a, h, o, c] for o in 0..2; o in 3..5 zeros
    a_i = np.arange(P)[:, None]
    c_i = np.arange(P)[None, :]
    rb_hoc = np.zeros((H, 3, P, P), np.float32)
    for o in range(3):
        dist = np.minimum(np.abs((o - 1) * P + c_i - a_i), REL_MAX)
        rb_hoc[:, o] = relb[:, dist] - relb[:, REL_MAX][:, None, None]
    rbz_a = np.zeros((P, H, 6, P), np.float32)
    rbz_a[:, :, 0:3, :] = 32.0 * rb_hoc.transpose(2, 0, 1, 3)
    rbz_a = f8(rbz_a)

    expd_a = np.zeros((4, 2, P), np.float32)
    for j in range(2):
        expd_a[j, j, 0:64] = 1.0
        expd_a[2 + j, j, 64:P] = 1.0
    expd_a = np.ascontiguousarray(expd_a.reshape(4, 2 * P))

    ident = np.eye(P, dtype=np.float32)
    shared = dict(
        wq8=wq_a, wk8=wk_a, wv8=wv_a, wo8=wo_a, w18=w1_a, w28=w2_a,
        bf1p=bf1p_a, qkb=qkb_a, lnw=lnw_a, rbz8=rbz_a,
        i32d=f8(ident * (2.0 ** -5)), identfd=ident,
        expd=expd_a, uvec4=np.ascontiguousarray(
            np.eye(4, dtype=np.float32).reshape(1, 16)),
        ones_v8=f8(np.concatenate(
            [np.ones((P, 8, 8, 1), np.float32),
             np.zeros((P, 8, 8, 15), np.float32)], axis=3)),
        zpad=np.zeros((64, 4, U), F8NP),
    )
    if bv_nz:
        shared["bvp"] = np.ascontiguousarray((32.0 * bv)[None, :])
    if bf2_nz:
        shared["bf2p"] = np.ascontiguousarray((2048.0 * bf2)[None, :])
    if not fast_gates:
        sid_a = np.zeros((P, H, P), np.float32)
        gid_a = np.zeros((P, H, 2, P), np.float32)
        for h in range(H):
            sid_a[:, h, :] = sims[h] * ident
            gid_a[:, h, 0, :] = -gate[h] * ident
        shared["sid8"] = f8(sid_a)
        shared["gid8"] = f8(gid_a)

    in_maps = []
    for b in range(B):
        Xb = X[b]
        validf = mask[b].astype(np.float32)
        norm = np.linalg.norm(Xb, axis=-1)
        rn = (1.0 / np.maximum(norm, 1e-6)) * validf
        Pmat = np.zeros((U, NCAT), np.float32)
        Pmat[np.arange(U), np.clip(spk[b], 0, NCAT - 1)] = 1.0
        ptb_a = np.zeros((NCAT, 2, U), np.float32)
        ptb_a[:, 0, :] = 16.0 * Pmat.T
        pta_a = np.zeros((NCAT, 2, U), np.float32)
        pta_a[:, 0, :] = (-16.0 * gate[0]) * Pmat.T
        m = dict(
            xt8=f8(Xb.T.reshape(4, P, U)),
            xpbo=np.ascontiguousarray(
                (1024.0 * (Xb + bo)).reshape(8, P, D)),
            rns_b=np.ascontiguousarray((16.0 * rn)[None, :]),
            pta2=f8(pta_a),
            ptb2=f8(ptb_a),
            validd=np.ascontiguousarray(validf.reshape(8, P).T),
            **shared,
        )
        if fast_gates:
            m["rns_a"] = np.ascontiguousarray(
                (16.0 * sims[0] * rn)[None, :])
        in_maps.append(m)

    res = run_bass_kernel_spmd(nc, in_maps, core_ids=list(range(NCORES)))
    outs = [r["out"].reshape(U, D) for r in res.results]
    return np.stack(outs).astype(np.float32)


# revision 43
# speedup vs baseline: 1.5467x; 1.0723x over previous
"""Trainium2 Bass kernel for nn_BiasedMHABlock (biased MHA + FFN transformer block).

Sharding: batch B=8 -> one batch per NeuronCore (SPMD, no collectives).

All heavy matmuls run as fp8e4 (e4m3) with MatmulPerfMode.DoubleRow: each
matmul instruction contracts two 128-row k-tiles (lhsT/rhs shaped [K,2,*])
at 0.5 PE cycles per output column -- 4x the fp32r FLOP rate.

Scale bookkeeping (all powers of two, exact):
  weights Wq(/8)/Wk/Wv/Wo/W1/W2 stored as 32x in fp8; X stored unscaled fp8.
  q,k true scale (activation scale 2^-5 on the 32x psum); vt = 32*v.
  scores psum is true scale: the per-head DoubleRow pairs (K_h | I/32) x
  (Q | 32*CB) and (I/32 | I/32) x (32*relband | 0) add the cosine/speaker
  bias CB and the banded relative-position bias inside the score matmul.
  et = exp(scores) in fp8; attn@V pairs w-tiles: (V 2-tiles | .) x (et pairs).
  attnT = 32*attn_out (fp8) -> Wo psum = 1024*(attn@Wo); xpbo = 1024*(X+bo);
  LN1 emits x1_t = 2048*x1 (f32, via rstd scale trick) and x1T = 32*x1 (fp8).
  W1 psum = 1024*z; hid = 64*relu(z) (fp8); W2 psum = 2048*ffn; residual add
  is scale-matched; LN2 normalization cancels the 2048 exactly.
Softmax runs over the partition axis without max-subtraction (scores are
O(1)); the denominator comes from an appended ones-column of V and is
divided out post-hoc (free columns of the same DoubleRow matmuls).
"""
import sys
import math

import os
for _p in ("/opt/trn_rl_repo", "/root/.axon_site/_ro/trn_rl_repo"):
    if os.path.isdir(_p) and _p not in sys.path:
        sys.path.insert(0, _p)

import numpy as np
import ml_dtypes

import concourse.bass as bass
import concourse.tile as tile
from concourse import bacc, mybir
from concourse.bass_utils import run_bass_kernel_spmd

F32 = mybir.dt.float32
F32R = mybir.dt.float32r
BF16 = mybir.dt.bfloat16
F8 = mybir.dt.float8e4
F8NP = mybir.dt.np(F8)
AF = mybir.ActivationFunctionType
ALU = mybir.AluOpType
PM = mybir.MatmulPerfMode

B, U, D, H, DH, DFF = 8, 1024, 512, 8, 64, 4096
REL_MAX = 128
P = 128
NCORES = 8
LN_EPS = 1e-5
NCAT = 16  # padded speaker-category partitions

_prog_cache = {}


def _drp(a0, a1):
    """DoubleRow pair AP from two same-shape 2-dim slices of one tile."""
    s = a1.offset - a0.offset
    return bass.AP(tensor=a0.tensor, offset=a0.offset,
                   ap=[list(a0.ap[0]), [s, 2], list(a0.ap[-1])])


def _build_program(fast_gates, apply_mask, ln1_triv, ln2_triv,
                   bf1_nz, bf2_nz, bv_nz):
    nc = bacc.Bacc("TRN2", target_bir_lowering=False, debug=False)

    def din(name, shape, dt=F8):
        return nc.dram_tensor(name, list(shape), dt, kind="ExternalInput").ap()

    xt8 = din("xt8", [4, P, U])
    xpbo = din("xpbo", [8, P, D], F32)
    rns_b = din("rns_b", [1, U], F32)
    if fast_gates:
        rns_a = din("rns_a", [1, U], F32)
    pta2 = din("pta2", [NCAT, 2, U])
    ptb2 = din("ptb2", [NCAT, 2, U])
    wq8 = din("wq8", [4, P, D])
    wk8 = din("wk8", [4, P, D])
    wv8 = din("wv8", [4, P, D])
    wo8 = din("wo8", [4, P, D])
    w18 = din("w18", [2, 4, P, DFF])   # fp8 residual split: [a;b] halves
    w28 = din("w28", [2, 32, P, D])
    qkb = din("qkb", [P, 8], F32)
    bf1p = din("bf1p", [P, 32], F32)
    rbz8 = din("rbz8", [P, H, 6, P])
    i32d = din("i32d", [P, P])            # I * 2^-5
    identfd = din("identfd", [P, P], F32)
    expd = din("expd", [4, 2 * P], F32R)
    uvec4 = din("uvec4", [1, 16], F32R)
    ones_v8 = din("ones_v8", [P, 8, 8, 16])
    zpad = din("zpad", [64, 4, U])        # fp8 zeros for kti pads
    lnw = din("lnw", [4, D], F32)
    validd = din("validd", [P, 8], F32)
    if bv_nz:
        bvp = din("bvp", [1, D], F32)     # 32*bv
    if bf2_nz:
        bf2p = din("bf2p", [1, D], F32)   # 2048*bf2
    if not fast_gates:
        sid8 = din("sid8", [P, H, P])     # sims[h] * I
        gid8 = din("gid8", [P, H, 2, P])  # [gate[h]*I ; 0]

    out = nc.dram_tensor("out", [8, P, D], F32, kind="ExternalOutput").ap()

    # qcb free-slot layout: 0:4 q packed, 4:12 cb (or sim), 12:20 eq(non-fast)
    NQ = 12 if fast_gates else 20
    open_cms = {}

    with tile.TileContext(nc) as tc, nc.allow_low_precision(reason="fp8 kernel"):
        def pool(name, bufs, space="SBUF", side="left"):
            cm = tc.tile_pool(name=name, bufs=bufs, space=space, side=side)
            p = cm.__enter__()
            open_cms[name] = cm
            return p

        def close(*names):
            for n in names:
                open_cms.pop(n).__exit__(None, None, None)

        try:
            # ---------------- constants (left, whole-kernel) ----------------
            consts = pool("consts", 1)
            identf = consts.tile([P, P], F32)
            idd = consts.tile([P, 2, P], F8)
            qkb_t = consts.tile([P, 8], F32)
            bf1_t = consts.tile([P, 32], F32)
            valid_t = consts.tile([P, 8], F32)
            eps1 = consts.tile([P, 1], F32)
            eps2 = consts.tile([P, 1], F32)

            # w2/xpbo preload pool: opened before attn_in (LIFO), DMAs issued
            # at phase-2 start so they overlap with attention compute
            w2p = pool("w2p", 1)
            w2_t = w2p.tile([P, 2, 32, D], F8, tag="w2", name="w2t")
            xpbo_t = w2p.tile([P, 8, D], F32, tag="xpbo", name="xpbot")
            w1_t = w2p.tile([P, 2, 4, DFF], F8, tag="w1", name="w1t")

            # ------------- long-lived attention inputs (left) ----------------
            attn_in = pool("attn_in", 1)
            kti = attn_in.tile([P, H, 1152], F8, tag="kti", name="kti")
            qcb = attn_in.tile([P, NQ, U], F8, tag="qcb", name="qcb")
            vt_t = attn_in.tile([P, 8, 640], F8, tag="vt", name="vt")
            rbz = attn_in.tile([P, H, 6, P], F8, tag="rbz", name="rbz")
            if not fast_gates:
                gid2 = attn_in.tile([P, H, 2, P], F8, tag="gid", name="gid")
                nc.sync.dma_start(gid2, gid8)

            # ======================= PHASE 1: prep ==========================
            pre = pool("pre", 1, side="right")
            prew = pool("prew", 3, side="right")
            ps1 = pool("ps1", 2, space="PSUM")

            xt_t = pre.tile([P, 4, U], F8, tag="xt", name="xtt")
            wq_t = prew.tile([P, 4, D], F8, tag="wx", name="wqt")
            wk_t = prew.tile([P, 4, D], F8, tag="wx", name="wkt")
            for c in range(4):
                nc.sync.dma_start(xt_t[:, c, :], xt8[c])
                nc.sync.dma_start(wq_t[:, c, :], wq8[c])
                nc.sync.dma_start(wk_t[:, c, :], wk8[c])
            nc.sync.dma_start(qkb_t, qkb)
            nc.sync.dma_start(identf, identfd)
            nc.sync.dma_start(
                idd, bass.AP(tensor=i32d.tensor, offset=0,
                             ap=[[P, P], [0, 2], [1, P]]))
            nc.vector.memset(eps1, LN_EPS / 4.0)
            nc.vector.memset(eps2, LN_EPS * float(2 ** 22))
            if bf1_nz:
                nc.sync.dma_start(bf1_t, bf1p)
            if apply_mask:
                nc.sync.dma_start(valid_t, validd)
            if not (ln1_triv and ln2_triv):
                lnwb = consts.tile([P, 4, D], F32)
                for k in range(4):
                    src = bass.AP(tensor=lnw.tensor, offset=k * D,
                                  ap=[[0, P], [1, D]])
                    nc.gpsimd.dma_start(lnwb[:, k, :], src)
            # kti pads: zero the complementary 64-partition halves
            _lo = kti[64:128, 0:1, 0:U]   # even-head slots, partitions 64..127
            nc.sync.dma_start(
                bass.AP(tensor=_lo.tensor, offset=_lo.offset,
                        ap=[list(_lo.ap[0]), [2 * 1152, 4], [1, U]]),
                zpad)
            _hi = kti[0:64, 1:2, 0:U]     # odd-head slots, partitions 0..63
            nc.sync.dma_start(
                bass.AP(tensor=_hi.tensor, offset=_hi.offset,
                        ap=[list(_hi.ap[0]), [2 * 1152, 4], [1, U]]),
                zpad)
            # kti ident region (per-head I/32, or sims[h]*I when not fast)
            if fast_gates:
                nc.sync.dma_start(
                    kti[:, :, 1024:1152],
                    bass.AP(tensor=i32d.tensor, offset=0,
                            ap=[[P, P], [0, H], [1, P]]))
            else:
                nc.sync.dma_start(kti[:, :, 1024:1152], sid8)

            rnsb_b = pre.tile([P, U], F32, tag="rnsb", name="rnsb")
            nc.gpsimd.dma_start(
                rnsb_b, bass.AP(tensor=rns_b.tensor, offset=0,
                                ap=[[0, P], [1, U]]))
            if fast_gates:
                rnsa_b = pre.tile([P, U], F32, tag="rnsa", name="rnsa")
                nc.gpsimd.dma_start(
                    rnsa_b, bass.AP(tensor=rns_a.tensor, offset=0,
                                    ap=[[0, P], [1, U]]))

            # Q (packed into qcb[:,0:4]) and K (padded per head in kti)
            for t in range(4):
                psq = ps1.tile([P, 2, D], F32, tag="psbig", name="psq")
                psk = ps1.tile([P, 2, D], F32, tag="psbig", name="psk")
                for j in range(2):
                    for cp in range(2):
                        nc.tensor.matmul(
                            psq[:, j, :],
                            wq_t[:, 2 * cp:2 * cp + 2, t * P:(t + 1) * P],
                            xt_t[:, 2 * cp:2 * cp + 2, j * D:(j + 1) * D],
                            start=(cp == 0), stop=(cp == 1),
                            perf_mode=PM.DoubleRow,
                        )
                        nc.tensor.matmul(
                            psk[:, j, :],
                            wk_t[:, 2 * cp:2 * cp + 2, t * P:(t + 1) * P],
                            xt_t[:, 2 * cp:2 * cp + 2, j * D:(j + 1) * D],
                            start=(cp == 0), stop=(cp == 1),
                            perf_mode=PM.DoubleRow,
                        )
                nc.scalar.activation(
                    qcb[:, t, :], psq[:, 0:2, :], AF.Identity,
                    bias=qkb_t[:, t:t + 1], scale=2.0 ** -5,
                )
                nc.scalar.activation(
                    kti[0:64, 2 * t, 0:U], psk[0:64, 0:2, :], AF.Identity,
                    bias=qkb_t[0:64, 4 + t:5 + t], scale=2.0 ** -5,
                )
                nc.scalar.activation(
                    kti[64:128, 2 * t + 1, 0:U], psk[64:128, 0:2, :],
                    AF.Identity,
                    bias=qkb_t[64:128, 4 + t:5 + t], scale=2.0 ** -5,
                )

            # V (interleaved (h dh) layout + ones cols); vt = 32*v
            wv_t = prew.tile([P, 4, D], F8, tag="wx", name="wvt")
            for c in range(4):
                nc.sync.dma_start(wv_t[:, c, :], wv8[c])
            nc.sync.dma_start(
                vt_t.rearrange("p t (h c) -> p t h c", c=80)[:, :, :, 64:80],
                ones_v8,
            )
            if bv_nz:
                bvp_b = pre.tile([P, D], F32, tag="bvp", name="bvp")
                nc.gpsimd.dma_start(
                    bvp_b, bass.AP(tensor=bvp.tensor, offset=0,
                                   ap=[[0, P], [1, D]]))
            for t in range(8):
                psv = ps1.tile([P, D], F32, tag="psv", name="psv")
                for cp in range(2):
                    nc.tensor.matmul(
                        psv,
                        xt_t[:, 2 * cp:2 * cp + 2, t * P:(t + 1) * P],
                        wv_t[:, 2 * cp:2 * cp + 2, :],
                        start=(cp == 0), stop=(cp == 1),
                        perf_mode=PM.DoubleRow,
                    )
                vdst = vt_t[:, t, :].rearrange(
                    "p (h c) -> p h c", c=80)[:, :, 0:64]
                vsrc = psv.rearrange("p (h dh) -> p h dh", h=H)
                if bv_nz:
                    nc.vector.tensor_tensor(
                        vdst, vsrc,
                        bvp_b.rearrange("p (h dh) -> p h dh", h=H), ALU.add)
                else:
                    nc.vector.tensor_copy(vdst, vsrc)
                if apply_mask:
                    nc.vector.tensor_scalar_mul(
                        vt_t[:, t, :], vt_t[:, t, :], valid_t[:, t:t + 1])

            # Xn tiles (16x scaled) and CB = 32*(sim_scale*XnXn^T - gate*SS^T)
            xnb_t = pre.tile([P, 4, U], F8, tag="xnb", name="xnb")
            for c in range(4):
                nc.gpsimd.tensor_tensor(
                    xnb_t[:, c, :], xt_t[:, c, :], rnsb_b, ALU.mult)
            if fast_gates:
                xna_t = pre.tile([P, 4, U], F8, tag="xna", name="xna")
                for c in range(4):
                    nc.gpsimd.tensor_tensor(
                        xna_t[:, c, :], xt_t[:, c, :], rnsa_b, ALU.mult)
            else:
                xna_t = xnb_t

            pta_t = pre.tile([NCAT, 2, U], F8, tag="pta", name="ptat")
            ptb_t = pre.tile([NCAT, 2, U], F8, tag="ptb", name="ptbt")
            nc.sync.dma_start(ptb_t, ptb2)
            if fast_gates:
                nc.sync.dma_start(pta_t, pta2)

            for i in range(8):
                pscb = ps1.tile([P, 2, D], F32, tag="psbig", name="pscb")
                for j in range(2):
                    for cp in range(2):
                        nc.tensor.matmul(
                            pscb[:, j, :],
                            xna_t[:, 2 * cp:2 * cp + 2, i * P:(i + 1) * P],
                            xnb_t[:, 2 * cp:2 * cp + 2, j * D:(j + 1) * D],
                            start=(cp == 0),
                            stop=(cp == 1 and not fast_gates),
                            perf_mode=PM.DoubleRow,
                        )
                    if fast_gates:
                        nc.tensor.matmul(
                            pscb[:, j, :],
                            pta_t[:, 0:2, i * P:(i + 1) * P],
                            ptb_t[:, 0:2, j * D:(j + 1) * D],
                            start=False, stop=True,
                            perf_mode=PM.DoubleRow,
                        )
                # fast: psum = 256*CB -> 32*CB ; nonfast: 256*sim -> sim
                nc.vector.tensor_scalar_mul(
                    qcb[:, 4 + i, :], pscb[:, 0:2, :],
                    (2.0 ** -3 if fast_gates else 2.0 ** -8),
                )
                if not fast_gates:
                    # eq indicator: 256*eq -> eq in qcb[:, 12+i]
                    pseq = ps1.tile([P, 2, D], F32, tag="psbig", name="pseq")
                    for j in range(2):
                        nc.tensor.matmul(
                            pseq[:, j, :],
                            ptb_t[:, 0:1, i * P:(i + 1) * P],
                            ptb_t[:, 0:1, j * D:(j + 1) * D],
                            start=True, stop=True,
                        )
                    nc.vector.tensor_scalar_mul(
                        qcb[:, 12 + i, :], pseq[:, 0:2, :], 2.0 ** -8)

            close("ps1", "prew", "pre")

            nc.sync.dma_start(rbz, rbz8)
            # preload FFN weights + xpbo on the idle Pool DGE queue so they
            # overlap attention compute without blocking the sync queue
            for c in range(32):
                nc.gpsimd.dma_start(w2_t[:, 0, c, :], w28[0, c])
                nc.gpsimd.dma_start(w2_t[:, 1, c, :], w28[1, c])
            for t in range(8):
                nc.gpsimd.dma_start(xpbo_t[:, t, :], xpbo[t])
            for hf in range(2):
                for c in range(4):
                    nc.gpsimd.dma_start(w1_t[:, hf, c, :], w18[hf, c])

            # ====================== PHASE 2: attention ======================
            mid = pool("mid", 1, side="right")
            attnT = mid.tile([P, 4, U], F8, tag="attnT", name="attnT")
            den_sb = mid.tile([1, 16, D], F32R, tag="densb", name="densb")
            expd_t = mid.tile([4, 2 * P], F32R, tag="expd", name="expdt")
            nc.sync.dma_start(expd_t, expd)
            uvec_t = mid.tile([1, 16], F32R, tag="uvec", name="uvect")
            nc.sync.dma_start(uvec_t, uvec4)

            epool = pool("epool", 2, side="right")
            arp = pool("arp", 2, side="right")
            dnp = pool("dnp", 4, side="right")
            pss = pool("pss", 2, space="PSUM")
            psa = pool("psa", 4, space="PSUM")

            rb_base = rbz[:, 0, 0, :]

            def rb_pair(h, o0, W):
                off0 = rb_base.offset + (h * 6 + o0) * P
                off1 = rb_base.offset + (h * 6 + 3) * P
                return bass.AP(tensor=rb_base.tensor, offset=off0,
                               ap=[list(rb_base.ap[0]), [off1 - off0, 2],
                                   [1, W]])

            for h in range(H):
                po = (h % 2) * 64
                ch = h // 2
                patts = [
                    psa.tile([65, D], F32, tag="psatt", name=f"psatt_{h}_{j}")
                    for j in range(2)
                ]
                if h % 2 == 0:
                    attnR = arp.tile([P, U], BF16, tag="attnR", name="attnR")
                for ip in range(4):
                    et = epool.tile([P, 2, U], F8, tag="et", name="et")
                    for ii in range(2):
                        i = 2 * ip + ii
                        ps = pss.tile([P, 2, D], F32, tag="pssc", name="pssc")
                        for j in range(2):
                            lhs1 = _drp(kti[:, h, i * P:(i + 1) * P],
                                        kti[:, h, 1024:1152])
                            rhs1 = _drp(qcb[:, ch, j * D:(j + 1) * D],
                                        qcb[:, 4 + i, j * D:(j + 1) * D])
                            lo_b = max(i - 1, 0)
                            hi_b = min(i + 1, 7)
                            run_lo = max(lo_b * P, j * D)
                            run_hi = min((hi_b + 1) * P, (j + 1) * D)
                            has_rel = run_hi > run_lo
                            if not fast_gates:
                                rhs_eq = bass.AP(
                                    tensor=qcb.tensor,
                                    offset=qcb[:, 12 + i,
                                               j * D:(j + 1) * D].offset,
                                    ap=[list(qcb[:, 12 + i, 0:1].ap[0]),
                                        [0, 2], [1, D]])
                            nc.tensor.matmul(
                                ps[:, j, :], lhs1, rhs1,
                                start=True,
                                stop=not (has_rel or not fast_gates),
                                perf_mode=PM.DoubleRow,
                                skip_group_check=True,
                            )
                            if not fast_gates:
                                nc.tensor.matmul(
                                    ps[:, j, :], gid2[:, h, 0:2, :], rhs_eq,
                                    start=False, stop=not has_rel,
                                    perf_mode=PM.DoubleRow,
                                    skip_group_check=True,
                                )
                            if has_rel:
                                o0 = (run_lo // P) - (i - 1)
                                W = run_hi - run_lo
                                nc.tensor.matmul(
                                    ps[:, j, run_lo - j * D:run_hi - j * D],
                                    idd[:, 0:2, :], rb_pair(h, o0, W),
                                    start=False, stop=True,
                                    perf_mode=PM.DoubleRow,
                                    skip_group_check=True,
                                )
                        nc.scalar.activation(
                            et[:, ii, :], ps[:, 0:2, :], AF.Exp)
                    for j in range(2):
                        nc.tensor.matmul(
                            patts[j],
                            vt_t[:, 2 * ip:2 * ip + 2, h * 80:h * 80 + 65],
                            et[:, 0:2, j * D:(j + 1) * D],
                            start=(ip == 0), stop=(ip == 3),
                            perf_mode=PM.DoubleRow,
                        )
                for j in range(2):
                    idx = h * 2 + j
                    nc.vector.tensor_copy(
                        den_sb[0:1, idx, :], patts[j][64:65, :])
                    nc.vector.tensor_copy(
                        attnR[po:po + 64, j * D:(j + 1) * D],
                        patts[j][0:64, :])
                if h % 2 == 1:
                    c4 = 4 * ch
                    psg = psa.tile([4, D], F32, tag="psatt", name=f"psg_{ch}")
                    for r in range(4):
                        nc.tensor.matmul(
                            psg,
                            uvec_t[0:1, r * 4:(r + 1) * 4],
                            den_sb[0:1, c4 + r, :],
                            start=(r == 0), stop=(r == 3),
                        )
                    rden4 = dnp.tile([4, D], F32R, tag="rden4", name="rden4")
                    nc.vector.reciprocal(rden4, psg)
                    for j in range(2):
                        psn = psa.tile([P, D], F32, tag="psatt",
                                       name=f"psn_{ch}_{j}")
                        nc.tensor.matmul(
                            psn, expd_t[:, j * P:(j + 1) * P], rden4,
                            start=True, stop=True,
                        )
                        nc.vector.tensor_tensor(
                            attnT[:, ch, j * D:(j + 1) * D],
                            attnR[:, j * D:(j + 1) * D],
                            psn, ALU.mult,
                        )

            close("psa", "pss", "dnp", "arp", "epool")
            close("attn_in")

            # ---------- x1 pool opens early on the left (outlives mid) -------
            x1p = pool("x1p", 1)
            x1_t = x1p.tile([P, 8, D], F32, tag="x1", name="x1")
            x1T_t = x1p.tile([P, 4, U], F8, tag="x1T", name="x1T")

            # ================== PHASE 3: X1 = LN1(X+bo+attn@Wo) =============
            x1w = pool("x1w", 1, side="right")
            lns = pool("lns", 8, side="right")
            psc = pool("psc", 3, space="PSUM")
            pst = pool("pst", 3, space="PSUM")

            wo_t = x1w.tile([P, 4, D], F8, tag="wo", name="wot")
            for c in range(4):
                nc.sync.dma_start(wo_t[:, c, :], wo8[c])

            for t in range(8):
                ps = psc.tile([P, D], F32, tag="psx1", name="psx1")
                for cp in range(2):
                    nc.tensor.matmul(
                        ps,
                        attnT[:, 2 * cp:2 * cp + 2, t * P:(t + 1) * P],
                        wo_t[:, 2 * cp:2 * cp + 2, :],
                        start=(cp == 0), stop=(cp == 1),
                        perf_mode=PM.DoubleRow,
                    )
                o1 = lns.tile([P, D], F32, tag="o1", name="o1")
                nc.vector.tensor_tensor(o1, ps, xpbo_t[:, t, :], ALU.add)
                stats = lns.tile([P, 6], F32, tag="st", name="st")
                nc.vector.bn_stats(stats, o1)
                mv = lns.tile([P, 2], F32, tag="mv", name="mv")
                nc.vector.bn_aggr(mv, stats)
                rstd = lns.tile([P, 1], F32, tag="rstd", name="rstd")
                nc.scalar.activation(rstd, mv[:, 1:2], AF.Sqrt,
                                     bias=eps1, scale=2.0 ** -22)
                nc.vector.reciprocal(rstd, rstd)
                # x1_t = 2048*x1
                if ln1_triv and not apply_mask:
                    nmr = lns.tile([P, 1], F32, tag="nmr", name="nmr")
                    nc.vector.tensor_scalar(
                        nmr, mv[:, 0:1], rstd, -1.0, ALU.mult, ALU.mult)
                    nc.scalar.activation(
                        x1_t[:, t, :], o1, AF.Identity,
                        bias=nmr, scale=rstd)
                else:
                    xh = lns.tile([P, D], F32, tag="xh", name="xh")
                    nc.vector.tensor_scalar(
                        xh, o1, mv[:, 0:1], rstd, ALU.subtract, ALU.mult)
                    if not ln1_triv:
                        xg = lns.tile([P, D], F32, tag="xg", name="xg")
                        nc.vector.tensor_tensor(xg, xh, lnwb[:, 0, :],
                                                ALU.mult)
                        nc.vector.tensor_tensor(xg, xg, lnwb[:, 1, :],
                                                ALU.add)
                        xh = xg
                    if apply_mask:
                        nc.vector.tensor_scalar_mul(
                            x1_t[:, t, :], xh, valid_t[:, t:t + 1])
                    else:
                        nc.vector.tensor_copy(x1_t[:, t, :], xh)
                for c in range(4):
                    pt = pst.tile([P, P], F32, tag="pstr", name="pstr")
                    nc.tensor.transpose(
                        pt, x1_t[:, t, c * P:(c + 1) * P], identf)
                    nc.scalar.activation(
                        x1T_t[:, c, t * P:(t + 1) * P], pt, AF.Copy,
                        scale=2.0 ** -6)

            close("pst", "psc", "lns", "x1w")
            close("mid")

            # ========================= PHASE 4: FFN =========================
            ffnw = pool("ffnw", 1)
            hidp = pool("hidp", 1, side="right")
            lns2 = pool("lns2", 8, side="right")
            outp = pool("outp", 2, side="right")
            psf = pool("psf", 4, space="PSUM")

            if bf2_nz:
                bf2_b = ffnw.tile([P, D], F32, tag="bf2b", name="bf2b")
                nc.gpsimd.dma_start(
                    bf2_b, bass.AP(tensor=bf2p.tensor, offset=0,
                                   ap=[[0, P], [1, D]]))

            ublk = 512
            for ub in range(U // ublk):
                hid = hidp.tile([P, 32, ublk], F8, tag="hid", name="hid")
                for t in range(32):
                    psh = psf.tile([P, ublk], F32, tag="psh", name="psh")
                    for hf in range(2):
                        for cp in range(2):
                            nc.tensor.matmul(
                                psh,
                                w1_t[:, hf, 2 * cp:2 * cp + 2,
                                     t * P:(t + 1) * P],
                                x1T_t[:, 2 * cp:2 * cp + 2,
                                      ub * ublk:(ub + 1) * ublk],
                                start=(hf == 0 and cp == 0),
                                stop=(hf == 1 and cp == 1),
                                perf_mode=PM.DoubleRow,
                            )
                    # hid = 64*relu(z); psum = 1024*z
                    if bf1_nz:
                        nc.scalar.activation(
                            hid[:, t, :], psh, AF.Relu,
                            bias=bf1_t[:, t:t + 1], scale=2.0 ** -10)
                    elif t % 4 == 0:
                        nc.vector.tensor_scalar(
                            hid[:, t, :], psh, 0.0, 2.0 ** -4,
                            ALU.max, ALU.mult)
                    else:
                        nc.scalar.activation(
                            hid[:, t, :], psh, AF.Relu, scale=2.0 ** -4)
                nv = ublk // P
                psos = [
                    psf.tile([P, D], F32, tag="pso", name=f"pso{v}")
                    for v in range(nv)
                ]
                for c2 in range(16):
                    for v in range(nv):
                        for hf in range(2):
                            nc.tensor.matmul(
                                psos[v],
                                hid[:, 2 * c2:2 * c2 + 2, v * P:(v + 1) * P],
                                w2_t[:, hf, 2 * c2:2 * c2 + 2, :],
                                start=(c2 == 0 and hf == 0),
                                stop=(c2 == 15 and hf == 1),
                                perf_mode=PM.DoubleRow,
                                skip_group_check=True,
                            )
                for v in range(nv):
                    g = ub * nv + v
                    x2p = lns2.tile([P, D], F32, tag="x2p", name="x2p")
                    nc.vector.tensor_tensor(x2p, psos[v], x1_t[:, g, :],
                                            ALU.add)
                    if bf2_nz:
                        nc.vector.tensor_tensor(x2p, x2p, bf2_b, ALU.add)
                    stats = lns2.tile([P, 6], F32, tag="st2", name="st2")
                    nc.vector.bn_stats(stats, x2p)
                    mv = lns2.tile([P, 2], F32, tag="mv2", name="mv2")
                    nc.vector.bn_aggr(mv, stats)
                    rstd = lns2.tile([P, 1], F32, tag="rstd2", name="rstd2")
                    nc.scalar.activation(rstd, mv[:, 1:2], AF.Sqrt,
                                         bias=eps2, scale=1.0)
                    nc.vector.reciprocal(rstd, rstd)
                    x2 = outp.tile([P, D], F32, tag="x2", name="x2")
                    if ln2_triv and not apply_mask:
                        nmr2 = lns2.tile([P, 1], F32, tag="nmr2", name="nmr2")
                        nc.vector.tensor_scalar(
                            nmr2, mv[:, 0:1], rstd, -1.0, ALU.mult, ALU.mult)
                        nc.scalar.activation(
                            x2, x2p, AF.Identity, bias=nmr2, scale=rstd)
                    else:
                        xh = lns2.tile([P, D], F32, tag="xh2", name="xh2")
                        nc.vector.tensor_scalar(
                            xh, x2p, mv[:, 0:1], rstd,
                            ALU.subtract, ALU.mult)
                        if not ln2_triv:
                            xg = lns2.tile([P, D], F32, tag="xg2", name="xg2")
                            nc.vector.tensor_tensor(xg, xh, lnwb[:, 2, :],
                                                    ALU.mult)
                            nc.vector.tensor_tensor(xg, xg, lnwb[:, 3, :],
                                                    ALU.add)
                            xh = xg
                        if apply_mask:
                            nc.vector.tensor_scalar_mul(
                                x2, xh, valid_t[:, g:g + 1])
                        else:
                            nc.vector.tensor_copy(x2, xh)
                    nc.sync.dma_start(out[g], x2)

            close("psf", "outp", "lns2", "hidp", "ffnw", "x1p",
                  "w2p", "consts")
        finally:
            for n in list(open_cms):
                try:
                    open_cms.pop(n).__exit__(None, None, None)
                except Exception:
                    pass

    nc.compile()
    return nc


def _get_program(*key):
    if key not in _prog_cache:
        _prog_cache[key] = _build_program(*key)
    return _prog_cache[key]


def kernel(**inputs):
    X = np.ascontiguousarray(np.asarray(inputs["X"], dtype=np.float32))
    mask = np.asarray(inputs["mask_u"]).astype(bool)
    spk = np.asarray(inputs["speakers"]).astype(np.int64)
    Wq = np.asarray(inputs["Wq"], np.float32); bq = np.asarray(inputs["bq"], np.float32)
    Wk = np.asarray(inputs["Wk"], np.float32); bk = np.asarray(inputs["bk"], np.float32)
    Wv = np.asarray(inputs["Wv"], np.float32); bv = np.asarray(inputs["bv"], np.float32)
    Wo = np.asarray(inputs["Wo"], np.float32); bo = np.asarray(inputs["bo"], np.float32)
    relb = np.asarray(inputs["rel_bias"], np.float32)
    gate = np.asarray(inputs["speaker_gate"], np.float32)
    sims = np.asarray(inputs["sim_scale"], np.float32)
    g1 = np.asarray(inputs["g1"], np.float32); beta1 = np.asarray(inputs["beta1"], np.float32)
    g2 = np.asarray(inputs["g2"], np.float32); beta2 = np.asarray(inputs["beta2"], np.float32)
    W1 = np.asarray(inputs["W1"], np.float32); bf1 = np.asarray(inputs["bf1"], np.float32)
    W2 = np.asarray(inputs["W2"], np.float32); bf2 = np.asarray(inputs["bf2"], np.float32)

    fast_gates = bool(np.all(gate == gate[0]) and np.all(sims == sims[0]))
    apply_mask = not bool(mask.all())
    ln1_triv = bool(np.all(g1 == 1.0) and np.all(beta1 == 0.0))
    ln2_triv = bool(np.all(g2 == 1.0) and np.all(beta2 == 0.0))
    bf1_nz = bool(np.any(bf1 != 0.0))
    bf2_nz = bool(np.any(bf2 != 0.0))
    bv_nz = bool(np.any(bv != 0.0))
    nc = _get_program(fast_gates, apply_mask, ln1_triv, ln2_triv,
                      bf1_nz, bf2_nz, bv_nz)

    f8 = lambda a: np.ascontiguousarray(a).astype(F8NP)
    scale = 1.0 / math.sqrt(DH)

    wq_a = f8((Wq * (scale * 32.0)).reshape(4, P, D))
    wk_a = f8((Wk * 32.0).reshape(4, P, D))
    wv_a = f8((Wv * 32.0).reshape(4, P, D))
    wo_a = f8((Wo * 32.0).reshape(4, P, D))
    def split8(w):
        a = w.astype(F8NP).astype(np.float32)
        b = (w - a).astype(F8NP).astype(np.float32)
        return np.stack([a, b]).astype(F8NP)

    w1_a = np.ascontiguousarray(split8((W1 * 32.0).reshape(4, P, DFF)))
    hid_scale = 1.0 if bf1_nz else 64.0
    w2_a = np.ascontiguousarray(
        split8((W2 * (2048.0 / hid_scale)).reshape(32, P, D)))
    bf1p_a = np.ascontiguousarray(bf1.reshape(32, P).T)
    qkb_a = np.zeros((P, 8), np.float32)
    qkb_a[:, 0:4] = (bq * scale).reshape(4, P).T
    qkb_a[:, 4:8] = bk.reshape(4, P).T
    lnw_a = np.ascontiguousarray(
        np.stack([g1, beta1 * 2048.0, g2, beta2]))

    # banded rel bias (32x): rbz[a, h, o, c] for o in 0..2; o in 3..5 zeros
    a_i = np.arange(P)[:, None]
    c_i = np.arange(P)[None, :]
    rb_hoc = np.zeros((H, 3, P, P), np.float32)
    for o in range(3):
        dist = np.minimum(np.abs((o - 1) * P + c_i - a_i), REL_MAX)
        rb_hoc[:, o] = relb[:, dist] - relb[:, REL_MAX][:, None, None]
    rbz_a = np.zeros((P, H, 6, P), np.float32)
    rbz_a[:, :, 0:3, :] = 32.0 * rb_hoc.transpose(2, 0, 1, 3)
    rbz_a = f8(rbz_a)

    expd_a = np.zeros((4, 2, P), np.float32)
    for j in range(2):
        expd_a[j, j, 0:64] = 1.0
        expd_a[2 + j, j, 64:P] = 1.0
    expd_a = np.ascontiguousarray(expd_a.reshape(4, 2 * P))

    ident = np.eye(P, dtype=np.float32)
    shared = dict(
        wq8=wq_a, wk8=wk_a, wv8=wv_a, wo8=wo_a, w18=w1_a, w28=w2_a,
        bf1p=bf1p_a, qkb=qkb_a, lnw=lnw_a, rbz8=rbz_a,
        i32d=f8(ident * (2.0 ** -5)), identfd=ident,
        expd=expd_a, uvec4=np.ascontiguousarray(
            np.eye(4, dtype=np.float32).reshape(1, 16)),
        ones_v8=f8(np.concatenate(
            [np.ones((P, 8, 8, 1), np.float32),
             np.zeros((P, 8, 8, 15), np.float32)], axis=3)),
        zpad=np.zeros((64, 4, U), F8NP),
    )
    if bv_nz:
        shared["bvp"] = np.ascontiguousarray((32.0 * bv)[None, :])
    if bf2_nz:
        shared["bf2p"] = np.ascontiguousarray((2048.0 * bf2)[None, :])
    if not fast_gates:
        sid_a = np.zeros((P, H, P), np.float32)
        gid_a = np.zeros((P, H, 2, P), np.float32)
        for h in range(H):
            sid_a[:, h, :] = sims[h] * ident
            gid_a[:, h, 0, :] = -gate[h] * ident
        shared["sid8"] = f8(sid_a)
        shared["gid8"] = f8(gid_a)

    in_maps = []
    for b in range(B):
        Xb = X[b]
        validf = mask[b].astype(np.float32)
        norm = np.linalg.norm(Xb, axis=-1)
        rn = (1.0 / np.maximum(norm, 1e-6)) * validf
        Pmat = np.zeros((U, NCAT), np.float32)
        Pmat[np.arange(U), np.clip(spk[b], 0, NCAT - 1)] = 1.0
        ptb_a = np.zeros((NCAT, 2, U), np.float32)
        ptb_a[:, 0, :] = 16.0 * Pmat.T
        pta_a = np.zeros((NCAT, 2, U), np.float32)
        pta_a[:, 0, :] = (-16.0 * gate[0]) * Pmat.T
        m = dict(
            xt8=f8(Xb.T.reshape(4, P, U)),
            xpbo=np.ascontiguousarray(
                (1024.0 * (Xb + bo)).reshape(8, P, D)),
            rns_b=np.ascontiguousarray((16.0 * rn)[None, :]),
            pta2=f8(pta_a),
            ptb2=f8(ptb_a),
            validd=np.ascontiguousarray(validf.reshape(8, P).T),
            **shared,
        )
        if fast_gates:
            m["rns_a"] = np.ascontiguousarray(
                (16.0 * sims[0] * rn)[None, :])
        in_maps.append(m)

    res = run_bass_kernel_spmd(nc, in_maps, core_ids=list(range(NCORES)))
    outs = [r["out"].reshape(U, D) for r in res.results]
    return np.stack(outs).astype(np.float32)


# revision 59
# speedup vs baseline: 1.5584x; 1.0076x over previous
"""Trainium2 Bass kernel for nn_BiasedMHABlock (biased MHA + FFN transformer block).

Sharding: batch B=8 -> one batch per NeuronCore (SPMD, no collectives).

All heavy matmuls run as fp8e4 (e4m3) with MatmulPerfMode.DoubleRow: each
matmul instruction contracts two 128-row k-tiles (lhsT/rhs shaped [K,2,*])
at 0.5 PE cycles per output column -- 4x the fp32r FLOP rate.

Scale bookkeeping (all powers of two, exact):
  weights Wq(/8)/Wk/Wv/Wo/W1/W2 stored as 32x in fp8; X stored unscaled fp8.
  q,k true scale (activation scale 2^-5 on the 32x psum); vt = 32*v.
  scores psum is true scale: the per-head DoubleRow pairs (K_h | I/32) x
  (Q | 32*CB) and (I/32 | I/32) x (32*relband | 0) add the cosine/speaker
  bias CB and the banded relative-position bias inside the score matmul.
  et = exp(scores) in fp8; attn@V pairs w-tiles: (V 2-tiles | .) x (et pairs).
  attnT = 32*attn_out (fp8) -> Wo psum = 1024*(attn@Wo); xpbo = 1024*(X+bo);
  LN1 emits x1_t = 2048*x1 (f32, via rstd scale trick) and x1T = 32*x1 (fp8).
  W1 psum = 1024*z; hid = 64*relu(z) (fp8); W2 psum = 2048*ffn; residual add
  is scale-matched; LN2 normalization cancels the 2048 exactly.
Softmax runs over the partition axis without max-subtraction (scores are
O(1)); the denominator comes from an appended ones-column of V and is
divided out post-hoc (free columns of the same DoubleRow matmuls).
"""
import sys
import math

import os
for _p in ("/opt/trn_rl_repo", "/root/.axon_site/_ro/trn_rl_repo"):
    if os.path.isdir(_p) and _p not in sys.path:
        sys.path.insert(0, _p)

import numpy as np
import ml_dtypes

import concourse.bass as bass
import concourse.tile as tile
from concourse import bacc, mybir
from concourse.bass_utils import run_bass_kernel_spmd

F32 = mybir.dt.float32
F32R = mybir.dt.float32r
BF16 = mybir.dt.bfloat16
F8 = mybir.dt.float8e4
F8NP = mybir.dt.np(F8)
AF = mybir.ActivationFunctionType
ALU = mybir.AluOpType
PM = mybir.MatmulPerfMode

B, U, D, H, DH, DFF = 8, 1024, 512, 8, 64, 4096
REL_MAX = 128
P = 128
NCORES = 8
LN_EPS = 1e-5
NCAT = 16  # padded speaker-category partitions

_prog_cache = {}


def _drp(a0, a1):
    """DoubleRow pair AP from two same-shape 2-dim slices of one tile."""
    s = a1.offset - a0.offset
    return bass.AP(tensor=a0.tensor, offset=a0.offset,
                   ap=[list(a0.ap[0]), [s, 2], list(a0.ap[-1])])


def _build_program(fast_gates, apply_mask, ln1_triv, ln2_triv,
                   bf1_nz, bf2_nz, bv_nz):
    nc = bacc.Bacc("TRN2", target_bir_lowering=False, debug=False)

    def din(name, shape, dt=F8):
        return nc.dram_tensor(name, list(shape), dt, kind="ExternalInput").ap()

    xt8 = din("xt8", [4, P, U])
    xpbo = din("xpbo", [8, P, D], BF16)
    rns_b = din("rns_b", [1, U], F32)
    if fast_gates:
        rns_a = din("rns_a", [1, U], F32)
    pta2 = din("pta2", [NCAT, 2, U])
    ptb2 = din("ptb2", [NCAT, 2, U])
    wq8 = din("wq8", [4, P, D])
    wk8 = din("wk8", [4, P, D])
    wv8 = din("wv8", [4, P, D])
    wo8 = din("wo8", [4, P, D])
    w18 = din("w18", [2, 4, P, DFF])   # fp8 residual split: [a;b] halves
    w28 = din("w28", [2, 32, P, D])
    qkb = din("qkb", [P, 8], F32)
    bf1p = din("bf1p", [P, 32], F32)
    rbz8 = din("rbz8", [P, H, 6, P])
    i32d = din("i32d", [P, P])            # I * 2^-5
    identfd = din("identfd", [P, P], F32)
    identbd = din("identbd", [P, P], BF16)
    expd = din("expd", [4, 2 * P], F32R)
    uvec4 = din("uvec4", [1, 16], F32R)
    ones_v8 = din("ones_v8", [P, 8, 8, 16])
    zpad = din("zpad", [64, 4, U])        # fp8 zeros for kti pads
    lnw = din("lnw", [4, D], F32)
    validd = din("validd", [P, 8], F32)
    if bv_nz:
        bvp = din("bvp", [1, D], F32)     # 32*bv
    if bf2_nz:
        bf2p = din("bf2p", [1, D], F32)   # 2048*bf2
    if not fast_gates:
        sid8 = din("sid8", [P, H, P])     # sims[h] * I
        gid8 = din("gid8", [P, H, 2, P])  # [gate[h]*I ; 0]

    out = nc.dram_tensor("out", [8, P, D], F32, kind="ExternalOutput").ap()

    # qcb free-slot layout: 0:4 q packed, 4:12 cb (or sim), 12:20 eq(non-fast)
    NQ = 12 if fast_gates else 20
    open_cms = {}

    with tile.TileContext(nc) as tc, nc.allow_low_precision(reason="fp8 kernel"):
        def pool(name, bufs, space="SBUF", side="left"):
            cm = tc.tile_pool(name=name, bufs=bufs, space=space, side=side)
            p = cm.__enter__()
            open_cms[name] = cm
            return p

        def close(*names):
            for n in names:
                open_cms.pop(n).__exit__(None, None, None)

        try:
            # ---------------- constants (left, whole-kernel) ----------------
            consts = pool("consts", 1)
            identf = consts.tile([P, P], F32)
            identb = consts.tile([P, P], BF16)
            identr = consts.tile([P, P], F32R)
            idd = consts.tile([P, 2, P], F8)
            qkb_t = consts.tile([P, 8], F32)
            bf1_t = consts.tile([P, 32], F32)
            valid_t = consts.tile([P, 8], F32)
            eps1 = consts.tile([P, 1], F32)
            eps2 = consts.tile([P, 1], F32)

            # w2/xpbo preload pool: opened before attn_in (LIFO), DMAs issued
            # at phase-2 start so they overlap with attention compute
            w2p = pool("w2p", 1)
            w2_t = w2p.tile([P, 2, 32, D], F8, tag="w2", name="w2t")
            xpbo_t = w2p.tile([P, 8, D], BF16, tag="xpbo", name="xpbot")
            w1_t = w2p.tile([P, 2, 4, DFF], F8, tag="w1", name="w1t")

            # ------------- long-lived attention inputs (left) ----------------
            attn_in = pool("attn_in", 1)
            kti = attn_in.tile([P, H, 1152], F8, tag="kti", name="kti")
            qcb = attn_in.tile([P, NQ, U], F8, tag="qcb", name="qcb")
            vt_t = attn_in.tile([P, 8, 640], F8, tag="vt", name="vt")
            rbz = attn_in.tile([P, H, 6, P], F8, tag="rbz", name="rbz")
            if not fast_gates:
                gid2 = attn_in.tile([P, H, 2, P], F8, tag="gid", name="gid")
                nc.sync.dma_start(gid2, gid8)

            # ======================= PHASE 1: prep ==========================
            pre = pool("pre", 1, side="right")
            prew = pool("prew", 3, side="right")
            ps1 = pool("ps1", 2, space="PSUM")

            xt_t = pre.tile([P, 4, U], F8, tag="xt", name="xtt")
            wq_t = prew.tile([P, 4, D], F8, tag="wx", name="wqt")
            wk_t = prew.tile([P, 4, D], F8, tag="wx", name="wkt")
            for c in range(4):
                nc.sync.dma_start(xt_t[:, c, :], xt8[c])
                nc.sync.dma_start(wq_t[:, c, :], wq8[c])
                nc.sync.dma_start(wk_t[:, c, :], wk8[c])
            nc.sync.dma_start(qkb_t, qkb)
            nc.sync.dma_start(identf, identfd)
            nc.sync.dma_start(identb, identbd)
            nc.gpsimd.dma_start(identr, identfd)
            nc.sync.dma_start(
                idd, bass.AP(tensor=i32d.tensor, offset=0,
                             ap=[[P, P], [0, 2], [1, P]]))
            nc.vector.memset(eps1, LN_EPS / 4.0)
            nc.vector.memset(eps2, LN_EPS * float(2 ** 22))
            if bf1_nz:
                nc.sync.dma_start(bf1_t, bf1p)
            if apply_mask:
                nc.sync.dma_start(valid_t, validd)
            if not (ln1_triv and ln2_triv):
                lnwb = consts.tile([P, 4, D], F32)
                for k in range(4):
                    src = bass.AP(tensor=lnw.tensor, offset=k * D,
                                  ap=[[0, P], [1, D]])
                    nc.gpsimd.dma_start(lnwb[:, k, :], src)
            # kti pads: zero the complementary 64-partition halves
            _lo = kti[64:128, 0:1, 0:U]   # even-head slots, partitions 64..127
            nc.sync.dma_start(
                bass.AP(tensor=_lo.tensor, offset=_lo.offset,
                        ap=[list(_lo.ap[0]), [2 * 1152, 4], [1, U]]),
                zpad)
            _hi = kti[0:64, 1:2, 0:U]     # odd-head slots, partitions 0..63
            nc.sync.dma_start(
                bass.AP(tensor=_hi.tensor, offset=_hi.offset,
                        ap=[list(_hi.ap[0]), [2 * 1152, 4], [1, U]]),
                zpad)
            # kti ident region (per-head I/32, or sims[h]*I when not fast)
            if fast_gates:
                nc.sync.dma_start(
                    kti[:, :, 1024:1152],
                    bass.AP(tensor=i32d.tensor, offset=0,
                            ap=[[P, P], [0, H], [1, P]]))
            else:
                nc.sync.dma_start(kti[:, :, 1024:1152], sid8)

            rnsb_b = pre.tile([P, U], F32, tag="rnsb", name="rnsb")
            nc.gpsimd.dma_start(
                rnsb_b, bass.AP(tensor=rns_b.tensor, offset=0,
                                ap=[[0, P], [1, U]]))
            if fast_gates:
                rnsa_b = pre.tile([P, U], F32, tag="rnsa", name="rnsa")
                nc.gpsimd.dma_start(
                    rnsa_b, bass.AP(tensor=rns_a.tensor, offset=0,
                                    ap=[[0, P], [1, U]]))

            # Q (packed into qcb[:,0:4]) and K (padded per head in kti)
            for t in range(4):
                psq = ps1.tile([P, 2, D], F32, tag="psbig", name="psq")
                psk = ps1.tile([P, 2, D], F32, tag="psbig", name="psk")
                for j in range(2):
                    for cp in range(2):
                        nc.tensor.matmul(
                            psq[:, j, :],
                            wq_t[:, 2 * cp:2 * cp + 2, t * P:(t + 1) * P],
                            xt_t[:, 2 * cp:2 * cp + 2, j * D:(j + 1) * D],
                            start=(cp == 0), stop=(cp == 1),
                            perf_mode=PM.DoubleRow,
                        )
                        nc.tensor.matmul(
                            psk[:, j, :],
                            wk_t[:, 2 * cp:2 * cp + 2, t * P:(t + 1) * P],
                            xt_t[:, 2 * cp:2 * cp + 2, j * D:(j + 1) * D],
                            start=(cp == 0), stop=(cp == 1),
                            perf_mode=PM.DoubleRow,
                        )
                nc.vector.tensor_scalar(
                    qcb[:, t, :], psq[:, 0:2, :], 2.0 ** -5,
                    qkb_t[:, t:t + 1], ALU.mult, ALU.add)
                nc.vector.tensor_scalar(
                    kti[0:64, 2 * t, 0:U], psk[0:64, 0:2, :], 2.0 ** -5,
                    qkb_t[0:64, 4 + t:5 + t], ALU.mult, ALU.add)
                nc.vector.tensor_scalar(
                    kti[64:128, 2 * t + 1, 0:U], psk[64:128, 0:2, :],
                    2.0 ** -5,
                    qkb_t[64:128, 4 + t:5 + t], ALU.mult, ALU.add)

            # V (interleaved (h dh) layout + ones cols); vt = 32*v
            wv_t = prew.tile([P, 4, D], F8, tag="wx", name="wvt")
            for c in range(4):
                nc.sync.dma_start(wv_t[:, c, :], wv8[c])
            nc.sync.dma_start(
                vt_t.rearrange("p t (h c) -> p t h c", c=80)[:, :, :, 64:80],
                ones_v8,
            )
            if bv_nz:
                bvp_b = pre.tile([P, D], F32, tag="bvp", name="bvp")
                nc.gpsimd.dma_start(
                    bvp_b, bass.AP(tensor=bvp.tensor, offset=0,
                                   ap=[[0, P], [1, D]]))
            for t in range(8):
                psv = ps1.tile([P, D], F32, tag="psv", name="psv")
                for cp in range(2):
                    nc.tensor.matmul(
                        psv,
                        xt_t[:, 2 * cp:2 * cp + 2, t * P:(t + 1) * P],
                        wv_t[:, 2 * cp:2 * cp + 2, :],
                        start=(cp == 0), stop=(cp == 1),
                        perf_mode=PM.DoubleRow,
                    )
                vdst = vt_t[:, t, :].rearrange(
                    "p (h c) -> p h c", c=80)[:, :, 0:64]
                vsrc = psv.rearrange("p (h dh) -> p h dh", h=H)
                if bv_nz:
                    nc.vector.tensor_tensor(
                        vdst, vsrc,
                        bvp_b.rearrange("p (h dh) -> p h dh", h=H), ALU.add)
                else:
                    nc.vector.tensor_copy(vdst, vsrc)
                if apply_mask:
                    nc.vector.tensor_scalar_mul(
                        vt_t[:, t, :], vt_t[:, t, :], valid_t[:, t:t + 1])

            # Xn tiles (16x scaled) and CB = 32*(sim_scale*XnXn^T - gate*SS^T)
            xnb_t = pre.tile([P, 4, U], F8, tag="xnb", name="xnb")
            for c in range(4):
                nc.gpsimd.tensor_tensor(
                    xnb_t[:, c, :], xt_t[:, c, :], rnsb_b, ALU.mult)
            if fast_gates:
                xna_t = pre.tile([P, 4, U], F8, tag="xna", name="xna")
                for c in range(4):
                    nc.gpsimd.tensor_tensor(
                        xna_t[:, c, :], xt_t[:, c, :], rnsa_b, ALU.mult)
            else:
                xna_t = xnb_t

            pta_t = pre.tile([NCAT, 2, U], F8, tag="pta", name="ptat")
            ptb_t = pre.tile([NCAT, 2, U], F8, tag="ptb", name="ptbt")
            nc.sync.dma_start(ptb_t, ptb2)
            if fast_gates:
                nc.sync.dma_start(pta_t, pta2)

            for i in range(8):
                pscb = ps1.tile([P, 2, D], F32, tag="psbig", name="pscb")
                for j in range(2):
                    for cp in range(2):
                        nc.tensor.matmul(
                            pscb[:, j, :],
                            xna_t[:, 2 * cp:2 * cp + 2, i * P:(i + 1) * P],
                            xnb_t[:, 2 * cp:2 * cp + 2, j * D:(j + 1) * D],
                            start=(cp == 0),
                            stop=(cp == 1 and not fast_gates),
                            perf_mode=PM.DoubleRow,
                        )
                    if fast_gates:
                        nc.tensor.matmul(
                            pscb[:, j, :],
                            pta_t[:, 0:2, i * P:(i + 1) * P],
                            ptb_t[:, 0:2, j * D:(j + 1) * D],
                            start=False, stop=True,
                            perf_mode=PM.DoubleRow,
                        )
                # fast: psum = 256*CB -> 32*CB ; nonfast: 256*sim -> sim
                nc.vector.tensor_scalar_mul(
                    qcb[:, 4 + i, :], pscb[:, 0:2, :],
                    (2.0 ** -3 if fast_gates else 2.0 ** -8),
                )
                if not fast_gates:
                    # eq indicator: 256*eq -> eq in qcb[:, 12+i]
                    pseq = ps1.tile([P, 2, D], F32, tag="psbig", name="pseq")
                    for j in range(2):
                        nc.tensor.matmul(
                            pseq[:, j, :],
                            ptb_t[:, 0:1, i * P:(i + 1) * P],
                            ptb_t[:, 0:1, j * D:(j + 1) * D],
                            start=True, stop=True,
                        )
                    nc.vector.tensor_scalar_mul(
                        qcb[:, 12 + i, :], pseq[:, 0:2, :], 2.0 ** -8)

            close("ps1", "prew", "pre")

            nc.sync.dma_start(rbz, rbz8)
            # preload FFN weights + xpbo on the idle Pool DGE queue so they
            # overlap attention compute without blocking the sync queue
            for c in range(32):
                nc.gpsimd.dma_start(w2_t[:, 0, c, :], w28[0, c])
                nc.gpsimd.dma_start(w2_t[:, 1, c, :], w28[1, c])
            for t in range(8):
                nc.gpsimd.dma_start(xpbo_t[:, t, :], xpbo[t])
            for hf in range(2):
                for c in range(4):
                    nc.gpsimd.dma_start(w1_t[:, hf, c, :], w18[hf, c])

            # ====================== PHASE 2: attention ======================
            mid = pool("mid", 1, side="right")
            attnT = mid.tile([P, 4, U], F8, tag="attnT", name="attnT")
            den_sb = mid.tile([1, 16, D], F32R, tag="densb", name="densb")
            expd_t = mid.tile([4, 2 * P], F32R, tag="expd", name="expdt")
            nc.sync.dma_start(expd_t, expd)
            uvec_t = mid.tile([1, 16], F32R, tag="uvec", name="uvect")
            nc.sync.dma_start(uvec_t, uvec4)

            epool = pool("epool", 2, side="right")
            arp = pool("arp", 2, side="right")
            dnp = pool("dnp", 4, side="right")
            pss = pool("pss", 2, space="PSUM")
            psa = pool("psa", 4, space="PSUM")

            rb_base = rbz[:, 0, 0, :]

            def rb_pair(h, o0, W):
                off0 = rb_base.offset + (h * 6 + o0) * P
                off1 = rb_base.offset + (h * 6 + 3) * P
                return bass.AP(tensor=rb_base.tensor, offset=off0,
                               ap=[list(rb_base.ap[0]), [off1 - off0, 2],
                                   [1, W]])

            for h in range(H):
                po = (h % 2) * 64
                ch = h // 2
                patts = [
                    psa.tile([65, D], F32, tag="psatt", name=f"psatt_{h}_{j}")
                    for j in range(2)
                ]
                if h % 2 == 0:
                    attnR = arp.tile([P, U], BF16, tag="attnR", name="attnR")
                for ip in range(4):
                    et = epool.tile([P, 2, U], F8, tag="et", name="et")
                    for ii in range(2):
                        i = 2 * ip + ii
                        ps = pss.tile([P, 2, D], F32, tag="pssc", name="pssc")
                        for j in range(2):
                            lhs1 = _drp(kti[:, h, i * P:(i + 1) * P],
                                        kti[:, h, 1024:1152])
                            rhs1 = _drp(qcb[:, ch, j * D:(j + 1) * D],
                                        qcb[:, 4 + i, j * D:(j + 1) * D])
                            lo_b = max(i - 1, 0)
                            hi_b = min(i + 1, 7)
                            run_lo = max(lo_b * P, j * D)
                            run_hi = min((hi_b + 1) * P, (j + 1) * D)
                            has_rel = run_hi > run_lo
                            if not fast_gates:
                                rhs_eq = bass.AP(
                                    tensor=qcb.tensor,
                                    offset=qcb[:, 12 + i,
                                               j * D:(j + 1) * D].offset,
                                    ap=[list(qcb[:, 12 + i, 0:1].ap[0]),
                                        [0, 2], [1, D]])
                            nc.tensor.matmul(
                                ps[:, j, :], lhs1, rhs1,
                                start=True,
                                stop=not (has_rel or not fast_gates),
                                perf_mode=PM.DoubleRow,
                                skip_group_check=True,
                            )
                            if not fast_gates:
                                nc.tensor.matmul(
                                    ps[:, j, :], gid2[:, h, 0:2, :], rhs_eq,
                                    start=False, stop=not has_rel,
                                    perf_mode=PM.DoubleRow,
                                    skip_group_check=True,
                                )
                            if has_rel:
                                o0 = (run_lo // P) - (i - 1)
                                W = run_hi - run_lo
                                nc.tensor.matmul(
                                    ps[:, j, run_lo - j * D:run_hi - j * D],
                                    idd[:, 0:2, :], rb_pair(h, o0, W),
                                    start=False, stop=True,
                                    perf_mode=PM.DoubleRow,
                                    skip_group_check=True,
                                )
                        nc.scalar.activation(
                            et[:, ii, :], ps[:, 0:2, :], AF.Exp)
                    for j in range(2):
                        nc.tensor.matmul(
                            patts[j],
                            vt_t[:, 2 * ip:2 * ip + 2, h * 80:h * 80 + 65],
                            et[:, 0:2, j * D:(j + 1) * D],
                            start=(ip == 0), stop=(ip == 3),
                            perf_mode=PM.DoubleRow,
                        )
                for j in range(2):
                    idx = h * 2 + j
                    nc.vector.tensor_copy(
                        den_sb[0:1, idx, :], patts[j][64:65, :])
                    nc.vector.tensor_copy(
                        attnR[po:po + 64, j * D:(j + 1) * D],
                        patts[j][0:64, :])
                if h % 2 == 1:
                    c4 = 4 * ch
                    psg = psa.tile([4, D], F32, tag="psatt", name=f"psg_{ch}")
                    for r in range(4):
                        nc.tensor.matmul(
                            psg,
                            uvec_t[0:1, r * 4:(r + 1) * 4],
                            den_sb[0:1, c4 + r, :],
                            start=(r == 0), stop=(r == 3),
                        )
                    rden4 = dnp.tile([4, D], F32R, tag="rden4", name="rden4")
                    nc.vector.reciprocal(rden4, psg)
                    for j in range(2):
                        psn = psa.tile([P, D], F32, tag="psatt",
                                       name=f"psn_{ch}_{j}")
                        nc.tensor.matmul(
                            psn, expd_t[:, j * P:(j + 1) * P], rden4,
                            start=True, stop=True,
                        )
                        nc.vector.tensor_tensor(
                            attnT[:, ch, j * D:(j + 1) * D],
                            attnR[:, j * D:(j + 1) * D],
                            psn, ALU.mult,
                        )

            close("psa", "pss", "dnp", "arp", "epool")
            close("attn_in")

            # ---------- x1 pool opens early on the left (outlives mid) -------
            x1p = pool("x1p", 1)
            x1_t = x1p.tile([P, 8, D], F32R, tag="x1", name="x1")
            x1T_t = x1p.tile([P, 4, U], F8, tag="x1T", name="x1T")

            # ================== PHASE 3: X1 = LN1(X+bo+attn@Wo) =============
            x1w = pool("x1w", 1, side="right")
            lns = pool("lns", 8, side="right")
            psc = pool("psc", 3, space="PSUM")
            pst = pool("pst", 3, space="PSUM")

            wo_t = x1w.tile([P, 4, D], F8, tag="wo", name="wot")
            for c in range(4):
                nc.sync.dma_start(wo_t[:, c, :], wo8[c])

            for t in range(8):
                ps = psc.tile([P, D], F32, tag="psx1", name="psx1")
                nc.tensor.matmul(
                    ps, identb, xpbo_t[:, t, :],
                    start=True, stop=False, skip_group_check=True,
                )
                for cp in range(2):
                    nc.tensor.matmul(
                        ps,
                        attnT[:, 2 * cp:2 * cp + 2, t * P:(t + 1) * P],
                        wo_t[:, 2 * cp:2 * cp + 2, :],
                        start=False, stop=(cp == 1),
                        perf_mode=PM.DoubleRow,
                        skip_group_check=True,
                    )
                o1 = ps
                stats = lns.tile([P, 6], F32, tag="st", name="st")
                nc.vector.bn_stats(stats, o1)
                mv = lns.tile([P, 2], F32, tag="mv", name="mv")
                nc.vector.bn_aggr(mv, stats)
                rstd = lns.tile([P, 1], F32, tag="rstd", name="rstd")
                nc.scalar.activation(rstd, mv[:, 1:2], AF.Sqrt,
                                     bias=eps1, scale=2.0 ** -22)
                nc.vector.reciprocal(rstd, rstd)
                # x1_t = 2048*x1
                if ln1_triv and not apply_mask:
                    nmr = lns.tile([P, 1], F32, tag="nmr", name="nmr")
                    nc.vector.tensor_scalar(
                        nmr, mv[:, 0:1], rstd, -1.0, ALU.mult, ALU.mult)
                    nc.scalar.activation(
                        x1_t[:, t, :], o1, AF.Identity,
                        bias=nmr, scale=rstd)
                else:
                    xh = lns.tile([P, D], F32, tag="xh", name="xh")
                    nc.vector.tensor_scalar(
                        xh, o1, mv[:, 0:1], rstd, ALU.subtract, ALU.mult)
                    if not ln1_triv:
                        xg = lns.tile([P, D], F32, tag="xg", name="xg")
                        nc.vector.tensor_tensor(xg, xh, lnwb[:, 0, :],
                                                ALU.mult)
                        nc.vector.tensor_tensor(xg, xg, lnwb[:, 1, :],
                                                ALU.add)
                        xh = xg
                    if apply_mask:
                        nc.vector.tensor_scalar_mul(
                            x1_t[:, t, :], xh, valid_t[:, t:t + 1])
                    else:
                        nc.vector.tensor_copy(x1_t[:, t, :], xh)
                for c in range(4):
                    pt = pst.tile([P, P], F32R, tag="pstr", name="pstr")
                    nc.tensor.transpose(
                        pt, x1_t[:, t, c * P:(c + 1) * P], identr)
                    nc.scalar.activation(
                        x1T_t[:, c, t * P:(t + 1) * P], pt, AF.Copy,
                        scale=2.0 ** -6)

            close("pst", "psc", "lns", "x1w")
            close("mid")

            # ========================= PHASE 4: FFN =========================
            ffnw = pool("ffnw", 1)
            hidp = pool("hidp", 1, side="right")
            lns2 = pool("lns2", 8, side="right")
            outp = pool("outp", 2, side="right")
            psf = pool("psf", 4, space="PSUM")

            if bf2_nz:
                bf2_b = ffnw.tile([P, D], F32, tag="bf2b", name="bf2b")
                nc.gpsimd.dma_start(
                    bf2_b, bass.AP(tensor=bf2p.tensor, offset=0,
                                   ap=[[0, P], [1, D]]))

            ublk = 512
            for ub in range(U // ublk):
                hid = hidp.tile([P, 32, ublk], F8, tag="hid", name="hid")
                for t in range(32):
                    psh = psf.tile([P, ublk], F32, tag="psh", name="psh")
                    for hf in range(2):
                        for cp in range(2):
                            nc.tensor.matmul(
                                psh,
                                w1_t[:, hf, 2 * cp:2 * cp + 2,
                                     t * P:(t + 1) * P],
                                x1T_t[:, 2 * cp:2 * cp + 2,
                                      ub * ublk:(ub + 1) * ublk],
                                start=(hf == 0 and cp == 0),
                                stop=(hf == 1 and cp == 1),
                                perf_mode=PM.DoubleRow,
                            )
                    # hid = 64*relu(z); psum = 1024*z
                    if bf1_nz:
                        nc.scalar.activation(
                            hid[:, t, :], psh, AF.Relu,
                            bias=bf1_t[:, t:t + 1], scale=2.0 ** -10)
                    elif t % 2 == 0:
                        nc.vector.tensor_scalar(
                            hid[:, t, :], psh, 0.0, 2.0 ** -4,
                            ALU.max, ALU.mult)
                    else:
                        nc.scalar.activation(
                            hid[:, t, :], psh, AF.Relu, scale=2.0 ** -4)
                nv = ublk // P
                psos = [
                    psf.tile([P, D], F32, tag="pso", name=f"pso{v}")
                    for v in range(nv)
                ]
                for v in range(nv):
                    nc.tensor.matmul(
                        psos[v], identr, x1_t[:, ub * nv + v, :],
                        start=True, stop=False, skip_group_check=True,
                    )
                for c2 in range(16):
                    for v in range(nv):
                        for hf in range(2):
                            nc.tensor.matmul(
                                psos[v],
                                hid[:, 2 * c2:2 * c2 + 2, v * P:(v + 1) * P],
                                w2_t[:, hf, 2 * c2:2 * c2 + 2, :],
                                start=False,
                                stop=(c2 == 15 and hf == 1),
                                perf_mode=PM.DoubleRow,
                                skip_group_check=True,
                            )
                for v in range(nv):
                    g = ub * nv + v
                    x2p = psos[v]
                    if bf2_nz:
                        x2b = lns2.tile([P, D], F32, tag="x2b", name="x2b")
                        nc.vector.tensor_tensor(x2b, x2p, bf2_b, ALU.add)
                        x2p = x2b
                    stats = lns2.tile([P, 6], F32, tag="st2", name="st2")
                    nc.vector.bn_stats(stats, x2p)
                    mv = lns2.tile([P, 2], F32, tag="mv2", name="mv2")
                    nc.vector.bn_aggr(mv, stats)
                    rstd = lns2.tile([P, 1], F32, tag="rstd2", name="rstd2")
                    nc.scalar.activation(rstd, mv[:, 1:2], AF.Sqrt,
                                         bias=eps2, scale=1.0)
                    nc.vector.reciprocal(rstd, rstd)
                    x2 = outp.tile([P, D], F32, tag="x2", name="x2")
                    if ln2_triv and not apply_mask:
                        nmr2 = lns2.tile([P, 1], F32, tag="nmr2", name="nmr2")
                        nc.vector.tensor_scalar(
                            nmr2, mv[:, 0:1], rstd, -1.0, ALU.mult, ALU.mult)
                        nc.scalar.activation(
                            x2, x2p, AF.Identity, bias=nmr2, scale=rstd)
                    else:
                        xh = lns2.tile([P, D], F32, tag="xh2", name="xh2")
                        nc.vector.tensor_scalar(
                            xh, x2p, mv[:, 0:1], rstd,
                            ALU.subtract, ALU.mult)
                        if not ln2_triv:
                            xg = lns2.tile([P, D], F32, tag="xg2", name="xg2")
                            nc.vector.tensor_tensor(xg, xh, lnwb[:, 2, :],
                                                    ALU.mult)
                            nc.vector.tensor_tensor(xg, xg, lnwb[:, 3, :],
                                                    ALU.add)
                            xh = xg
                        if apply_mask:
                            nc.vector.tensor_scalar_mul(
                                x2, xh, valid_t[:, g:g + 1])
                        else:
                            nc.vector.tensor_copy(x2, xh)
                    nc.sync.dma_start(out[g], x2)

            close("psf", "outp", "lns2", "hidp", "ffnw", "x1p",
                  "w2p", "consts")
        finally:
            for n in list(open_cms):
                try:
                    open_cms.pop(n).__exit__(None, None, None)
                except Exception:
                    pass

    nc.compile()
    return nc


def _get_program(*key):
    if key not in _prog_cache:
        _prog_cache[key] = _build_program(*key)
    return _prog_cache[key]


def kernel(**inputs):
    X = np.ascontiguousarray(np.asarray(inputs["X"], dtype=np.float32))
    mask = np.asarray(inputs["mask_u"]).astype(bool)
    spk = np.asarray(inputs["speakers"]).astype(np.int64)
    Wq = np.asarray(inputs["Wq"], np.float32); bq = np.asarray(inputs["bq"], np.float32)
    Wk = np.asarray(inputs["Wk"], np.float32); bk = np.asarray(inputs["bk"], np.float32)
    Wv = np.asarray(inputs["Wv"], np.float32); bv = np.asarray(inputs["bv"], np.float32)
    Wo = np.asarray(inputs["Wo"], np.float32); bo = np.asarray(inputs["bo"], np.float32)
    relb = np.asarray(inputs["rel_bias"], np.float32)
    gate = np.asarray(inputs["speaker_gate"], np.float32)
    sims = np.asarray(inputs["sim_scale"], np.float32)
    g1 = np.asarray(inputs["g1"], np.float32); beta1 = np.asarray(inputs["beta1"], np.float32)
    g2 = np.asarray(inputs["g2"], np.float32); beta2 = np.asarray(inputs["beta2"], np.float32)
    W1 = np.asarray(inputs["W1"], np.float32); bf1 = np.asarray(inputs["bf1"], np.float32)
    W2 = np.asarray(inputs["W2"], np.float32); bf2 = np.asarray(inputs["bf2"], np.float32)

    fast_gates = bool(np.all(gate == gate[0]) and np.all(sims == sims[0]))
    apply_mask = not bool(mask.all())
    ln1_triv = bool(np.all(g1 == 1.0) and np.all(beta1 == 0.0))
    ln2_triv = bool(np.all(g2 == 1.0) and np.all(beta2 == 0.0))
    bf1_nz = bool(np.any(bf1 != 0.0))
    bf2_nz = bool(np.any(bf2 != 0.0))
    bv_nz = bool(np.any(bv != 0.0))
    nc = _get_program(fast_gates, apply_mask, ln1_triv, ln2_triv,
                      bf1_nz, bf2_nz, bv_nz)

    f8 = lambda a: np.ascontiguousarray(a).astype(F8NP)
    scale = 1.0 / math.sqrt(DH)

    wq_a = f8((Wq * (scale * 32.0)).reshape(4, P, D))
    wk_a = f8((Wk * 32.0).reshape(4, P, D))
    wv_a = f8((Wv * 32.0).reshape(4, P, D))
    wo_a = f8((Wo * 32.0).reshape(4, P, D))
    def split8(w):
        a = w.astype(F8NP).astype(np.float32)
        b = (w - a).astype(F8NP).astype(np.float32)
        return np.stack([a, b]).astype(F8NP)

    w1_a = np.ascontiguousarray(split8((W1 * 32.0).reshape(4, P, DFF)))
    hid_scale = 1.0 if bf1_nz else 64.0
    w2_a = np.ascontiguousarray(
        split8((W2 * (2048.0 / hid_scale)).reshape(32, P, D)))
    bf1p_a = np.ascontiguousarray(bf1.reshape(32, P).T)
    qkb_a = np.zeros((P, 8), np.float32)
    qkb_a[:, 0:4] = (bq * scale).reshape(4, P).T
    qkb_a[:, 4:8] = bk.reshape(4, P).T
    lnw_a = np.ascontiguousarray(
        np.stack([g1, beta1 * 2048.0, g2, beta2]))

    # banded rel bias (32x): rbz[a, h, o, c] for o in 0..2; o in 3..5 zeros
    a_i = np.arange(P)[:, None]
    c_i = np.arange(P)[None, :]
    rb_hoc = np.zeros((H, 3, P, P), np.float32)
    for o in range(3):
        dist = np.minimum(np.abs((o - 1) * P + c_i - a_i), REL_MAX)
        rb_hoc[:, o] = relb[:, dist] - relb[:, REL_MAX][:, None, None]
    rbz_a = np.zeros((P, H, 6, P), np.float32)
    rbz_a[:, :, 0:3, :] = 32.0 * rb_hoc.transpose(2, 0, 1, 3)
    rbz_a = f8(rbz_a)

    expd_a = np.zeros((4, 2, P), np.float32)
    for j in range(2):
        expd_a[j, j, 0:64] = 1.0
        expd_a[2 + j, j, 64:P] = 1.0
    expd_a = np.ascontiguousarray(expd_a.reshape(4, 2 * P))

    ident = np.eye(P, dtype=np.float32)
    shared = dict(
        wq8=wq_a, wk8=wk_a, wv8=wv_a, wo8=wo_a, w18=w1_a, w28=w2_a,
        bf1p=bf1p_a, qkb=qkb_a, lnw=lnw_a, rbz8=rbz_a,
        i32d=f8(ident * (2.0 ** -5)), identfd=ident,
        identbd=ident.astype(ml_dtypes.bfloat16),
        expd=expd_a, uvec4=np.ascontiguousarray(
            np.eye(4, dtype=np.float32).reshape(1, 16)),
        ones_v8=f8(np.concatenate(
            [np.ones((P, 8, 8, 1), np.float32),
             np.zeros((P, 8, 8, 15), np.float32)], axis=3)),
        zpad=np.zeros((64, 4, U), F8NP),
    )
    if bv_nz:
        shared["bvp"] = np.ascontiguousarray((32.0 * bv)[None, :])
    if bf2_nz:
        shared["bf2p"] = np.ascontiguousarray((2048.0 * bf2)[None, :])
    if not fast_gates:
        sid_a = np.zeros((P, H, P), np.float32)
        gid_a = np.zeros((P, H, 2, P), np.float32)
        for h in range(H):
            sid_a[:, h, :] = sims[h] * ident
            gid_a[:, h, 0, :] = -gate[h] * ident
        shared["sid8"] = f8(sid_a)
        shared["gid8"] = f8(gid_a)

    in_maps = []
    for b in range(B):
        Xb = X[b]
        validf = mask[b].astype(np.float32)
        norm = np.linalg.norm(Xb, axis=-1)
        rn = (1.0 / np.maximum(norm, 1e-6)) * validf
        Pmat = np.zeros((U, NCAT), np.float32)
        Pmat[np.arange(U), np.clip(spk[b], 0, NCAT - 1)] = 1.0
        ptb_a = np.zeros((NCAT, 2, U), np.float32)
        ptb_a[:, 0, :] = 16.0 * Pmat.T
        pta_a = np.zeros((NCAT, 2, U), np.float32)
        pta_a[:, 0, :] = (-16.0 * gate[0]) * Pmat.T
        m = dict(
            xt8=f8(Xb.T.reshape(4, P, U)),
            xpbo=np.ascontiguousarray(
                (1024.0 * (Xb + bo)).reshape(8, P, D)).astype(
                    ml_dtypes.bfloat16),
            rns_b=np.ascontiguousarray((16.0 * rn)[None, :]),
            pta2=f8(pta_a),
            ptb2=f8(ptb_a),
            validd=np.ascontiguousarray(validf.reshape(8, P).T),
            **shared,
        )
        if fast_gates:
            m["rns_a"] = np.ascontiguousarray(
                (16.0 * sims[0] * rn)[None, :])
        in_maps.append(m)

    res = run_bass_kernel_spmd(nc, in_maps, core_ids=list(range(NCORES)))
    outs = [r["out"].reshape(U, D) for r in res.results]
    return np.stack(outs).astype(np.float32)


# revision 60
# speedup vs baseline: 1.5749x; 1.0106x over previous
"""Trainium2 Bass kernel for nn_BiasedMHABlock (biased MHA + FFN transformer block).

Sharding: batch B=8 -> one batch per NeuronCore (SPMD, no collectives).

All heavy matmuls run as fp8e4 (e4m3) with MatmulPerfMode.DoubleRow: each
matmul instruction contracts two 128-row k-tiles (lhsT/rhs shaped [K,2,*])
at 0.5 PE cycles per output column -- 4x the fp32r FLOP rate.

Scale bookkeeping (all powers of two, exact):
  weights Wq(/8)/Wk/Wv/Wo/W1/W2 stored as 32x in fp8; X stored unscaled fp8.
  q,k true scale (activation scale 2^-5 on the 32x psum); vt = 32*v.
  scores psum is true scale: the per-head DoubleRow pairs (K_h | I/32) x
  (Q | 32*CB) and (I/32 | I/32) x (32*relband | 0) add the cosine/speaker
  bias CB and the banded relative-position bias inside the score matmul.
  et = exp(scores) in fp8; attn@V pairs w-tiles: (V 2-tiles | .) x (et pairs).
  attnT = 32*attn_out (fp8) -> Wo psum = 1024*(attn@Wo); xpbo = 1024*(X+bo);
  LN1 emits x1_t = 2048*x1 (f32, via rstd scale trick) and x1T = 32*x1 (fp8).
  W1 psum = 1024*z; hid = 64*relu(z) (fp8); W2 psum = 2048*ffn; residual add
  is scale-matched; LN2 normalization cancels the 2048 exactly.
Softmax runs over the partition axis without max-subtraction (scores are
O(1)); the denominator comes from an appended ones-column of V and is
divided out post-hoc (free columns of the same DoubleRow matmuls).
"""
import sys
import math

import os
for _p in ("/opt/trn_rl_repo", "/root/.axon_site/_ro/trn_rl_repo"):
    if os.path.isdir(_p) and _p not in sys.path:
        sys.path.insert(0, _p)

import numpy as np
import ml_dtypes

import concourse.bass as bass
import concourse.tile as tile
from concourse import bacc, mybir
from concourse.bass_utils import run_bass_kernel_spmd

F32 = mybir.dt.float32
F32R = mybir.dt.float32r
BF16 = mybir.dt.bfloat16
F8 = mybir.dt.float8e4
F8NP = mybir.dt.np(F8)
AF = mybir.ActivationFunctionType
ALU = mybir.AluOpType
PM = mybir.MatmulPerfMode

B, U, D, H, DH, DFF = 8, 1024, 512, 8, 64, 4096
REL_MAX = 128
P = 128
NCORES = 8
LN_EPS = 1e-5
NCAT = 16  # padded speaker-category partitions

_prog_cache = {}


def _drp(a0, a1):
    """DoubleRow pair AP from two same-shape 2-dim slices of one tile."""
    s = a1.offset - a0.offset
    return bass.AP(tensor=a0.tensor, offset=a0.offset,
                   ap=[list(a0.ap[0]), [s, 2], list(a0.ap[-1])])


def _build_program(fast_gates, apply_mask, ln1_triv, ln2_triv,
                   bf1_nz, bf2_nz, bv_nz):
    nc = bacc.Bacc("TRN2", target_bir_lowering=False, debug=False)

    def din(name, shape, dt=F8):
        return nc.dram_tensor(name, list(shape), dt, kind="ExternalInput").ap()

    xt8 = din("xt8", [4, P, U])
    xpbo = din("xpbo", [8, P, D], BF16)
    rns_b = din("rns_b", [1, U], F32)
    if fast_gates:
        rns_a = din("rns_a", [1, U], F32)
    pta2 = din("pta2", [NCAT, 2, U])
    ptb2 = din("ptb2", [NCAT, 2, U])
    wq8 = din("wq8", [4, P, D])
    wk8 = din("wk8", [4, P, D])
    wv8 = din("wv8", [4, P, D])
    wo8 = din("wo8", [4, P, D])
    w18 = din("w18", [2, 4, P, DFF])   # fp8 residual split: [a;b] halves
    w28 = din("w28", [2, 32, P, D])
    qkb = din("qkb", [P, 8], F32)
    bf1p = din("bf1p", [P, 32], F32)
    rbz8 = din("rbz8", [P, H, 6, P])
    i32d = din("i32d", [P, P])            # I * 2^-5
    identfd = din("identfd", [P, P], F32)
    identbd = din("identbd", [P, P], BF16)
    expd = din("expd", [4, 2 * P], F32R)
    uvec4 = din("uvec4", [1, 16], F32R)
    ones_v8 = din("ones_v8", [P, 8, 8, 16])
    zpad = din("zpad", [64, 4, U])        # fp8 zeros for kti pads
    lnw = din("lnw", [4, D], F32)
    validd = din("validd", [P, 8], F32)
    if bv_nz:
        bvp = din("bvp", [1, D], F32)     # 32*bv
    if bf2_nz:
        bf2p = din("bf2p", [1, D], F32)   # 2048*bf2
    if not fast_gates:
        sid8 = din("sid8", [P, H, P])     # sims[h] * I
        gid8 = din("gid8", [P, H, 2, P])  # [gate[h]*I ; 0]

    out = nc.dram_tensor("out", [8, P, D], F32, kind="ExternalOutput").ap()

    # qcb free-slot layout: 0:4 q packed, 4:12 cb (or sim), 12:20 eq(non-fast)
    NQ = 12 if fast_gates else 20
    open_cms = {}

    with tile.TileContext(nc) as tc, nc.allow_low_precision(reason="fp8 kernel"):
        def pool(name, bufs, space="SBUF", side="left"):
            cm = tc.tile_pool(name=name, bufs=bufs, space=space, side=side)
            p = cm.__enter__()
            open_cms[name] = cm
            return p

        def close(*names):
            for n in names:
                open_cms.pop(n).__exit__(None, None, None)

        try:
            # ---------------- constants (left, whole-kernel) ----------------
            consts = pool("consts", 1)
            identf = consts.tile([P, P], F32)
            identb = consts.tile([P, P], BF16)
            identr = consts.tile([P, P], F32R)
            idd = consts.tile([P, 2, P], F8)
            qkb_t = consts.tile([P, 8], F32)
            bf1_t = consts.tile([P, 32], F32)
            valid_t = consts.tile([P, 8], F32)
            eps1 = consts.tile([P, 1], F32)
            eps2 = consts.tile([P, 1], F32)

            # w2/xpbo preload pool: opened before attn_in (LIFO), DMAs issued
            # at phase-2 start so they overlap with attention compute
            w2p = pool("w2p", 1)
            w2_t = w2p.tile([P, 2, 32, D], F8, tag="w2", name="w2t")
            xpbo_t = w2p.tile([P, 8, D], BF16, tag="xpbo", name="xpbot")
            w1_t = w2p.tile([P, 2, 4, DFF], F8, tag="w1", name="w1t")

            # ------------- long-lived attention inputs (left) ----------------
            attn_in = pool("attn_in", 1)
            kti = attn_in.tile([P, H, 1152], F8, tag="kti", name="kti")
            qcb = attn_in.tile([P, NQ, U], F8, tag="qcb", name="qcb")
            vt_t = attn_in.tile([P, 8, 640], F8, tag="vt", name="vt")
            rbz = attn_in.tile([P, H, 6, P], F8, tag="rbz", name="rbz")
            if not fast_gates:
                gid2 = attn_in.tile([P, H, 2, P], F8, tag="gid", name="gid")
                nc.sync.dma_start(gid2, gid8)

            # ======================= PHASE 1: prep ==========================
            pre = pool("pre", 1, side="right")
            prew = pool("prew", 3, side="right")
            ps1 = pool("ps1", 2, space="PSUM")

            xt_t = pre.tile([P, 4, U], F8, tag="xt", name="xtt")
            wq_t = prew.tile([P, 4, D], F8, tag="wx", name="wqt")
            wk_t = prew.tile([P, 4, D], F8, tag="wx", name="wkt")
            for c in range(4):
                nc.sync.dma_start(xt_t[:, c, :], xt8[c])
                nc.sync.dma_start(wq_t[:, c, :], wq8[c])
                nc.sync.dma_start(wk_t[:, c, :], wk8[c])
            nc.sync.dma_start(qkb_t, qkb)
            nc.sync.dma_start(identf, identfd)
            nc.sync.dma_start(identb, identbd)
            nc.gpsimd.dma_start(identr, identfd)
            nc.sync.dma_start(
                idd, bass.AP(tensor=i32d.tensor, offset=0,
                             ap=[[P, P], [0, 2], [1, P]]))
            nc.vector.memset(eps1, LN_EPS / 4.0)
            nc.vector.memset(eps2, LN_EPS * float(2 ** 22))
            if bf1_nz:
                nc.sync.dma_start(bf1_t, bf1p)
            if apply_mask:
                nc.sync.dma_start(valid_t, validd)
            if not (ln1_triv and ln2_triv):
                lnwb = consts.tile([P, 4, D], F32)
                for k in range(4):
                    src = bass.AP(tensor=lnw.tensor, offset=k * D,
                                  ap=[[0, P], [1, D]])
                    nc.gpsimd.dma_start(lnwb[:, k, :], src)
            # kti pads: zero the complementary 64-partition halves
            _lo = kti[64:128, 0:1, 0:U]   # even-head slots, partitions 64..127
            nc.sync.dma_start(
                bass.AP(tensor=_lo.tensor, offset=_lo.offset,
                        ap=[list(_lo.ap[0]), [2 * 1152, 4], [1, U]]),
                zpad)
            _hi = kti[0:64, 1:2, 0:U]     # odd-head slots, partitions 0..63
            nc.sync.dma_start(
                bass.AP(tensor=_hi.tensor, offset=_hi.offset,
                        ap=[list(_hi.ap[0]), [2 * 1152, 4], [1, U]]),
                zpad)
            # kti ident region (per-head I/32, or sims[h]*I when not fast)
            if fast_gates:
                nc.sync.dma_start(
                    kti[:, :, 1024:1152],
                    bass.AP(tensor=i32d.tensor, offset=0,
                            ap=[[P, P], [0, H], [1, P]]))
            else:
                nc.sync.dma_start(kti[:, :, 1024:1152], sid8)

            rnsb_b = pre.tile([P, U], F32, tag="rnsb", name="rnsb")
            nc.gpsimd.dma_start(
                rnsb_b, bass.AP(tensor=rns_b.tensor, offset=0,
                                ap=[[0, P], [1, U]]))
            if fast_gates:
                rnsa_b = pre.tile([P, U], F32, tag="rnsa", name="rnsa")
                nc.gpsimd.dma_start(
                    rnsa_b, bass.AP(tensor=rns_a.tensor, offset=0,
                                    ap=[[0, P], [1, U]]))

            # Q (packed into qcb[:,0:4]) and K (padded per head in kti)
            for t in range(4):
                psq = ps1.tile([P, 2, D], F32, tag="psbig", name="psq")
                psk = ps1.tile([P, 2, D], F32, tag="psbig", name="psk")
                for j in range(2):
                    for cp in range(2):
                        nc.tensor.matmul(
                            psq[:, j, :],
                            wq_t[:, 2 * cp:2 * cp + 2, t * P:(t + 1) * P],
                            xt_t[:, 2 * cp:2 * cp + 2, j * D:(j + 1) * D],
                            start=(cp == 0), stop=(cp == 1),
                            perf_mode=PM.DoubleRow,
                        )
                        nc.tensor.matmul(
                            psk[:, j, :],
                            wk_t[:, 2 * cp:2 * cp + 2, t * P:(t + 1) * P],
                            xt_t[:, 2 * cp:2 * cp + 2, j * D:(j + 1) * D],
                            start=(cp == 0), stop=(cp == 1),
                            perf_mode=PM.DoubleRow,
                        )
                nc.scalar.activation(
                    qcb[:, t, :], psq[:, 0:2, :], AF.Identity,
                    bias=qkb_t[:, t:t + 1], scale=2.0 ** -5,
                )
                nc.scalar.activation(
                    kti[0:64, 2 * t, 0:U], psk[0:64, 0:2, :], AF.Identity,
                    bias=qkb_t[0:64, 4 + t:5 + t], scale=2.0 ** -5,
                )
                nc.vector.tensor_scalar(
                    kti[64:128, 2 * t + 1, 0:U], psk[64:128, 0:2, :],
                    2.0 ** -5,
                    qkb_t[64:128, 4 + t:5 + t], ALU.mult, ALU.add)

            # V (interleaved (h dh) layout + ones cols); vt = 32*v
            wv_t = prew.tile([P, 4, D], F8, tag="wx", name="wvt")
            for c in range(4):
                nc.sync.dma_start(wv_t[:, c, :], wv8[c])
            nc.sync.dma_start(
                vt_t.rearrange("p t (h c) -> p t h c", c=80)[:, :, :, 64:80],
                ones_v8,
            )
            if bv_nz:
                bvp_b = pre.tile([P, D], F32, tag="bvp", name="bvp")
                nc.gpsimd.dma_start(
                    bvp_b, bass.AP(tensor=bvp.tensor, offset=0,
                                   ap=[[0, P], [1, D]]))
            for t in range(8):
                psv = ps1.tile([P, D], F32, tag="psv", name="psv")
                for cp in range(2):
                    nc.tensor.matmul(
                        psv,
                        xt_t[:, 2 * cp:2 * cp + 2, t * P:(t + 1) * P],
                        wv_t[:, 2 * cp:2 * cp + 2, :],
                        start=(cp == 0), stop=(cp == 1),
                        perf_mode=PM.DoubleRow,
                    )
                vdst = vt_t[:, t, :].rearrange(
                    "p (h c) -> p h c", c=80)[:, :, 0:64]
                vsrc = psv.rearrange("p (h dh) -> p h dh", h=H)
                if bv_nz:
                    nc.vector.tensor_tensor(
                        vdst, vsrc,
                        bvp_b.rearrange("p (h dh) -> p h dh", h=H), ALU.add)
                else:
                    nc.vector.tensor_copy(vdst, vsrc)
                if apply_mask:
                    nc.vector.tensor_scalar_mul(
                        vt_t[:, t, :], vt_t[:, t, :], valid_t[:, t:t + 1])

            # Xn tiles (16x scaled) and CB = 32*(sim_scale*XnXn^T - gate*SS^T)
            xnb_t = pre.tile([P, 4, U], F8, tag="xnb", name="xnb")
            for c in range(4):
                nc.gpsimd.tensor_tensor(
                    xnb_t[:, c, :], xt_t[:, c, :], rnsb_b, ALU.mult)
            if fast_gates:
                xna_t = pre.tile([P, 4, U], F8, tag="xna", name="xna")
                for c in range(4):
                    nc.gpsimd.tensor_tensor(
                        xna_t[:, c, :], xt_t[:, c, :], rnsa_b, ALU.mult)
            else:
                xna_t = xnb_t

            pta_t = pre.tile([NCAT, 2, U], F8, tag="pta", name="ptat")
            ptb_t = pre.tile([NCAT, 2, U], F8, tag="ptb", name="ptbt")
            nc.sync.dma_start(ptb_t, ptb2)
            if fast_gates:
                nc.sync.dma_start(pta_t, pta2)

            for i in range(8):
                pscb = ps1.tile([P, 2, D], F32, tag="psbig", name="pscb")
                for j in range(2):
                    for cp in range(2):
                        nc.tensor.matmul(
                            pscb[:, j, :],
                            xna_t[:, 2 * cp:2 * cp + 2, i * P:(i + 1) * P],
                            xnb_t[:, 2 * cp:2 * cp + 2, j * D:(j + 1) * D],
                            start=(cp == 0),
                            stop=(cp == 1 and not fast_gates),
                            perf_mode=PM.DoubleRow,
                        )
                    if fast_gates:
                        nc.tensor.matmul(
                            pscb[:, j, :],
                            pta_t[:, 0:2, i * P:(i + 1) * P],
                            ptb_t[:, 0:2, j * D:(j + 1) * D],
                            start=False, stop=True,
                            perf_mode=PM.DoubleRow,
                        )
                # fast: psum = 256*CB -> 32*CB ; nonfast: 256*sim -> sim
                nc.vector.tensor_scalar_mul(
                    qcb[:, 4 + i, :], pscb[:, 0:2, :],
                    (2.0 ** -3 if fast_gates else 2.0 ** -8),
                )
                if not fast_gates:
                    # eq indicator: 256*eq -> eq in qcb[:, 12+i]
                    pseq = ps1.tile([P, 2, D], F32, tag="psbig", name="pseq")
                    for j in range(2):
                        nc.tensor.matmul(
                            pseq[:, j, :],
                            ptb_t[:, 0:1, i * P:(i + 1) * P],
                            ptb_t[:, 0:1, j * D:(j + 1) * D],
                            start=True, stop=True,
                        )
                    nc.vector.tensor_scalar_mul(
                        qcb[:, 12 + i, :], pseq[:, 0:2, :], 2.0 ** -8)

            close("ps1", "prew", "pre")

            nc.sync.dma_start(rbz, rbz8)
            # preload FFN weights + xpbo on the idle Pool DGE queue so they
            # overlap attention compute without blocking the sync queue
            for c in range(32):
                nc.gpsimd.dma_start(w2_t[:, 0, c, :], w28[0, c])
                nc.gpsimd.dma_start(w2_t[:, 1, c, :], w28[1, c])
            for t in range(8):
                nc.gpsimd.dma_start(xpbo_t[:, t, :], xpbo[t])
            for hf in range(2):
                for c in range(4):
                    nc.gpsimd.dma_start(w1_t[:, hf, c, :], w18[hf, c])

            # ====================== PHASE 2: attention ======================
            mid = pool("mid", 1, side="right")
            attnT = mid.tile([P, 4, U], F8, tag="attnT", name="attnT")
            den_sb = mid.tile([1, 16, D], F32R, tag="densb", name="densb")
            expd_t = mid.tile([4, 2 * P], F32R, tag="expd", name="expdt")
            nc.sync.dma_start(expd_t, expd)
            uvec_t = mid.tile([1, 16], F32R, tag="uvec", name="uvect")
            nc.sync.dma_start(uvec_t, uvec4)

            epool = pool("epool", 2, side="right")
            arp = pool("arp", 2, side="right")
            dnp = pool("dnp", 4, side="right")
            pss = pool("pss", 2, space="PSUM")
            psa = pool("psa", 4, space="PSUM")

            rb_base = rbz[:, 0, 0, :]

            def rb_pair(h, o0, W):
                off0 = rb_base.offset + (h * 6 + o0) * P
                off1 = rb_base.offset + (h * 6 + 3) * P
                return bass.AP(tensor=rb_base.tensor, offset=off0,
                               ap=[list(rb_base.ap[0]), [off1 - off0, 2],
                                   [1, W]])

            for h in range(H):
                po = (h % 2) * 64
                ch = h // 2
                patts = [
                    psa.tile([65, D], F32, tag="psatt", name=f"psatt_{h}_{j}")
                    for j in range(2)
                ]
                if h % 2 == 0:
                    attnR = arp.tile([P, U], BF16, tag="attnR", name="attnR")
                for ip in range(4):
                    et = epool.tile([P, 2, U], F8, tag="et", name="et")
                    for ii in range(2):
                        i = 2 * ip + ii
                        ps = pss.tile([P, 2, D], F32, tag="pssc", name="pssc")
                        for j in range(2):
                            lhs1 = _drp(kti[:, h, i * P:(i + 1) * P],
                                        kti[:, h, 1024:1152])
                            rhs1 = _drp(qcb[:, ch, j * D:(j + 1) * D],
                                        qcb[:, 4 + i, j * D:(j + 1) * D])
                            lo_b = max(i - 1, 0)
                            hi_b = min(i + 1, 7)
                            run_lo = max(lo_b * P, j * D)
                            run_hi = min((hi_b + 1) * P, (j + 1) * D)
                            has_rel = run_hi > run_lo
                            if not fast_gates:
                                rhs_eq = bass.AP(
                                    tensor=qcb.tensor,
                                    offset=qcb[:, 12 + i,
                                               j * D:(j + 1) * D].offset,
                                    ap=[list(qcb[:, 12 + i, 0:1].ap[0]),
                                        [0, 2], [1, D]])
                            nc.tensor.matmul(
                                ps[:, j, :], lhs1, rhs1,
                                start=True,
                                stop=not (has_rel or not fast_gates),
                                perf_mode=PM.DoubleRow,
                                skip_group_check=True,
                            )
                            if not fast_gates:
                                nc.tensor.matmul(
                                    ps[:, j, :], gid2[:, h, 0:2, :], rhs_eq,
                                    start=False, stop=not has_rel,
                                    perf_mode=PM.DoubleRow,
                                    skip_group_check=True,
                                )
                            if has_rel:
                                o0 = (run_lo // P) - (i - 1)
                                W = run_hi - run_lo
                                nc.tensor.matmul(
                                    ps[:, j, run_lo - j * D:run_hi - j * D],
                                    idd[:, 0:2, :], rb_pair(h, o0, W),
                                    start=False, stop=True,
                                    perf_mode=PM.DoubleRow,
                                    skip_group_check=True,
                                )
                        nc.scalar.activation(
                            et[:, ii, :], ps[:, 0:2, :], AF.Exp)
                    for j in range(2):
                        nc.tensor.matmul(
                            patts[j],
                            vt_t[:, 2 * ip:2 * ip + 2, h * 80:h * 80 + 65],
                            et[:, 0:2, j * D:(j + 1) * D],
                            start=(ip == 0), stop=(ip == 3),
                            perf_mode=PM.DoubleRow,
                        )
                for j in range(2):
                    idx = h * 2 + j
                    nc.vector.tensor_copy(
                        den_sb[0:1, idx, :], patts[j][64:65, :])
                    nc.vector.tensor_copy(
                        attnR[po:po + 64, j * D:(j + 1) * D],
                        patts[j][0:64, :])
                if h % 2 == 1:
                    c4 = 4 * ch
                    psg = psa.tile([4, D], F32, tag="psatt", name=f"psg_{ch}")
                    for r in range(4):
                        nc.tensor.matmul(
                            psg,
                            uvec_t[0:1, r * 4:(r + 1) * 4],
                            den_sb[0:1, c4 + r, :],
                            start=(r == 0), stop=(r == 3),
                        )
                    rden4 = dnp.tile([4, D], F32R, tag="rden4", name="rden4")
                    nc.vector.reciprocal(rden4, psg)
                    for j in range(2):
                        psn = psa.tile([P, D], F32, tag="psatt",
                                       name=f"psn_{ch}_{j}")
                        nc.tensor.matmul(
                            psn, expd_t[:, j * P:(j + 1) * P], rden4,
                            start=True, stop=True,
                        )
                        nc.vector.tensor_tensor(
                            attnT[:, ch, j * D:(j + 1) * D],
                            attnR[:, j * D:(j + 1) * D],
                            psn, ALU.mult,
                        )

            close("psa", "pss", "dnp", "arp", "epool")
            close("attn_in")

            # ---------- x1 pool opens early on the left (outlives mid) -------
            x1p = pool("x1p", 1)
            x1_t = x1p.tile([P, 8, D], F32R, tag="x1", name="x1")
            x1T_t = x1p.tile([P, 4, U], F8, tag="x1T", name="x1T")

            # ================== PHASE 3: X1 = LN1(X+bo+attn@Wo) =============
            x1w = pool("x1w", 1, side="right")
            lns = pool("lns", 8, side="right")
            psc = pool("psc", 3, space="PSUM")
            pst = pool("pst", 3, space="PSUM")

            wo_t = x1w.tile([P, 4, D], F8, tag="wo", name="wot")
            for c in range(4):
                nc.sync.dma_start(wo_t[:, c, :], wo8[c])

            for t in range(8):
                ps = psc.tile([P, D], F32, tag="psx1", name="psx1")
                nc.tensor.matmul(
                    ps, identb, xpbo_t[:, t, :],
                    start=True, stop=False, skip_group_check=True,
                )
                for cp in range(2):
                    nc.tensor.matmul(
                        ps,
                        attnT[:, 2 * cp:2 * cp + 2, t * P:(t + 1) * P],
                        wo_t[:, 2 * cp:2 * cp + 2, :],
                        start=False, stop=(cp == 1),
                        perf_mode=PM.DoubleRow,
                        skip_group_check=True,
                    )
                o1 = ps
                stats = lns.tile([P, 6], F32, tag="st", name="st")
                nc.vector.bn_stats(stats, o1)
                mv = lns.tile([P, 2], F32, tag="mv", name="mv")
                nc.vector.bn_aggr(mv, stats)
                rstd = lns.tile([P, 1], F32, tag="rstd", name="rstd")
                nc.scalar.activation(rstd, mv[:, 1:2], AF.Sqrt,
                                     bias=eps1, scale=2.0 ** -22)
                nc.vector.reciprocal(rstd, rstd)
                # x1_t = 2048*x1
                if ln1_triv and not apply_mask:
                    nmr = lns.tile([P, 1], F32, tag="nmr", name="nmr")
                    nc.vector.tensor_scalar(
                        nmr, mv[:, 0:1], rstd, -1.0, ALU.mult, ALU.mult)
                    nc.scalar.activation(
                        x1_t[:, t, :], o1, AF.Identity,
                        bias=nmr, scale=rstd)
                else:
                    xh = lns.tile([P, D], F32, tag="xh", name="xh")
                    nc.vector.tensor_scalar(
                        xh, o1, mv[:, 0:1], rstd, ALU.subtract, ALU.mult)
                    if not ln1_triv:
                        xg = lns.tile([P, D], F32, tag="xg", name="xg")
                        nc.vector.tensor_tensor(xg, xh, lnwb[:, 0, :],
                                                ALU.mult)
                        nc.vector.tensor_tensor(xg, xg, lnwb[:, 1, :],
                                                ALU.add)
                        xh = xg
                    if apply_mask:
                        nc.vector.tensor_scalar_mul(
                            x1_t[:, t, :], xh, valid_t[:, t:t + 1])
                    else:
                        nc.vector.tensor_copy(x1_t[:, t, :], xh)
                for c in range(4):
                    pt = pst.tile([P, P], F32R, tag="pstr", name="pstr")
                    nc.tensor.transpose(
                        pt, x1_t[:, t, c * P:(c + 1) * P], identr)
                    nc.scalar.activation(
                        x1T_t[:, c, t * P:(t + 1) * P], pt, AF.Copy,
                        scale=2.0 ** -6)

            close("pst", "psc", "lns", "x1w")
            close("mid")

            # ========================= PHASE 4: FFN =========================
            ffnw = pool("ffnw", 1)
            hidp = pool("hidp", 1, side="right")
            lns2 = pool("lns2", 8, side="right")
            outp = pool("outp", 2, side="right")
            psf = pool("psf", 4, space="PSUM")

            if bf2_nz:
                bf2_b = ffnw.tile([P, D], F32, tag="bf2b", name="bf2b")
                nc.gpsimd.dma_start(
                    bf2_b, bass.AP(tensor=bf2p.tensor, offset=0,
                                   ap=[[0, P], [1, D]]))

            ublk = 512
            for ub in range(U // ublk):
                hid = hidp.tile([P, 32, ublk], F8, tag="hid", name="hid")
                for t in range(32):
                    psh = psf.tile([P, ublk], F32, tag="psh", name="psh")
                    for hf in range(2):
                        for cp in range(2):
                            nc.tensor.matmul(
                                psh,
                                w1_t[:, hf, 2 * cp:2 * cp + 2,
                                     t * P:(t + 1) * P],
                                x1T_t[:, 2 * cp:2 * cp + 2,
                                      ub * ublk:(ub + 1) * ublk],
                                start=(hf == 0 and cp == 0),
                                stop=(hf == 1 and cp == 1),
                                perf_mode=PM.DoubleRow,
                            )
                    # hid = 64*relu(z); psum = 1024*z
                    if bf1_nz:
                        nc.scalar.activation(
                            hid[:, t, :], psh, AF.Relu,
                            bias=bf1_t[:, t:t + 1], scale=2.0 ** -10)
                    elif t % 2 == 0:
                        nc.vector.tensor_scalar(
                            hid[:, t, :], psh, 0.0, 2.0 ** -4,
                            ALU.max, ALU.mult)
                    else:
                        nc.scalar.activation(
                            hid[:, t, :], psh, AF.Relu, scale=2.0 ** -4)
                nv = ublk // P
                psos = [
                    psf.tile([P, D], F32, tag="pso", name=f"pso{v}")
                    for v in range(nv)
                ]
                for v in range(nv):
                    nc.tensor.matmul(
                        psos[v], identr, x1_t[:, ub * nv + v, :],
                        start=True, stop=False, skip_group_check=True,
                    )
                for c2 in range(16):
                    for v in range(nv):
                        for hf in range(2):
                            nc.tensor.matmul(
                                psos[v],
                                hid[:, 2 * c2:2 * c2 + 2, v * P:(v + 1) * P],
                                w2_t[:, hf, 2 * c2:2 * c2 + 2, :],
                                start=False,
                                stop=(c2 == 15 and hf == 1),
                                perf_mode=PM.DoubleRow,
                                skip_group_check=True,
                            )
                for v in range(nv):
                    g = ub * nv + v
                    x2p = psos[v]
                    if bf2_nz:
                        x2b = lns2.tile([P, D], F32, tag="x2b", name="x2b")
                        nc.vector.tensor_tensor(x2b, x2p, bf2_b, ALU.add)
                        x2p = x2b
                    stats = lns2.tile([P, 6], F32, tag="st2", name="st2")
                    nc.vector.bn_stats(stats, x2p)
                    mv = lns2.tile([P, 2], F32, tag="mv2", name="mv2")
                    nc.vector.bn_aggr(mv, stats)
                    rstd = lns2.tile([P, 1], F32, tag="rstd2", name="rstd2")
                    nc.scalar.activation(rstd, mv[:, 1:2], AF.Sqrt,
                                         bias=eps2, scale=1.0)
                    nc.vector.reciprocal(rstd, rstd)
                    x2 = outp.tile([P, D], F32, tag="x2", name="x2")
                    if ln2_triv and not apply_mask:
                        nmr2 = lns2.tile([P, 1], F32, tag="nmr2", name="nmr2")
                        nc.vector.tensor_scalar(
                            nmr2, mv[:, 0:1], rstd, -1.0, ALU.mult, ALU.mult)
                        nc.scalar.activation(
                            x2, x2p, AF.Identity, bias=nmr2, scale=rstd)
                    else:
                        xh = lns2.tile([P, D], F32, tag="xh2", name="xh2")
                        nc.vector.tensor_scalar(
                            xh, x2p, mv[:, 0:1], rstd,
                            ALU.subtract, ALU.mult)
                        if not ln2_triv:
                            xg = lns2.tile([P, D], F32, tag="xg2", name="xg2")
                            nc.vector.tensor_tensor(xg, xh, lnwb[:, 2, :],
                                                    ALU.mult)
                            nc.vector.tensor_tensor(xg, xg, lnwb[:, 3, :],
                                                    ALU.add)
                            xh = xg
                        if apply_mask:
                            nc.vector.tensor_scalar_mul(
                                x2, xh, valid_t[:, g:g + 1])
                        else:
                            nc.vector.tensor_copy(x2, xh)
                    nc.sync.dma_start(out[g], x2)

            close("psf", "outp", "lns2", "hidp", "ffnw", "x1p",
                  "w2p", "consts")
        finally:
            for n in list(open_cms):
                try:
                    open_cms.pop(n).__exit__(None, None, None)
                except Exception:
                    pass

    nc.compile()
    return nc


def _get_program(*key):
    if key not in _prog_cache:
        _prog_cache[key] = _build_program(*key)
    return _prog_cache[key]


def kernel(**inputs):
    X = np.ascontiguousarray(np.asarray(inputs["X"], dtype=np.float32))
    mask = np.asarray(inputs["mask_u"]).astype(bool)
    spk = np.asarray(inputs["speakers"]).astype(np.int64)
    Wq = np.asarray(inputs["Wq"], np.float32); bq = np.asarray(inputs["bq"], np.float32)
    Wk = np.asarray(inputs["Wk"], np.float32); bk = np.asarray(inputs["bk"], np.float32)
    Wv = np.asarray(inputs["Wv"], np.float32); bv = np.asarray(inputs["bv"], np.float32)
    Wo = np.asarray(inputs["Wo"], np.float32); bo = np.asarray(inputs["bo"], np.float32)
    relb = np.asarray(inputs["rel_bias"], np.float32)
    gate = np.asarray(inputs["speaker_gate"], np.float32)
    sims = np.asarray(inputs["sim_scale"], np.float32)
    g1 = np.asarray(inputs["g1"], np.float32); beta1 = np.asarray(inputs["beta1"], np.float32)
    g2 = np.asarray(inputs["g2"], np.float32); beta2 = np.asarray(inputs["beta2"], np.float32)
    W1 = np.asarray(inputs["W1"], np.float32); bf1 = np.asarray(inputs["bf1"], np.float32)
    W2 = np.asarray(inputs["W2"], np.float32); bf2 = np.asarray(inputs["bf2"], np.float32)

    fast_gates = bool(np.all(gate == gate[0]) and np.all(sims == sims[0]))
    apply_mask = not bool(mask.all())
    ln1_triv = bool(np.all(g1 == 1.0) and np.all(beta1 == 0.0))
    ln2_triv = bool(np.all(g2 == 1.0) and np.all(beta2 == 0.0))
    bf1_nz = bool(np.any(bf1 != 0.0))
    bf2_nz = bool(np.any(bf2 != 0.0))
    bv_nz = bool(np.any(bv != 0.0))
    nc = _get_program(fast_gates, apply_mask, ln1_triv, ln2_triv,
                      bf1_nz, bf2_nz, bv_nz)

    f8 = lambda a: np.ascontiguousarray(a).astype(F8NP)
    scale = 1.0 / math.sqrt(DH)

    wq_a = f8((Wq * (scale * 32.0)).reshape(4, P, D))
    wk_a = f8((Wk * 32.0).reshape(4, P, D))
    wv_a = f8((Wv * 32.0).reshape(4, P, D))
    wo_a = f8((Wo * 32.0).reshape(4, P, D))
    def split8(w):
        a = w.astype(F8NP).astype(np.float32)
        b = (w - a).astype(F8NP).astype(np.float32)
        return np.stack([a, b]).astype(F8NP)

    w1_a = np.ascontiguousarray(split8((W1 * 32.0).reshape(4, P, DFF)))
    hid_scale = 1.0 if bf1_nz else 64.0
    w2_a = np.ascontiguousarray(
        split8((W2 * (2048.0 / hid_scale)).reshape(32, P, D)))
    bf1p_a = np.ascontiguousarray(bf1.reshape(32, P).T)
    qkb_a = np.zeros((P, 8), np.float32)
    qkb_a[:, 0:4] = (bq * scale).reshape(4, P).T
    qkb_a[:, 4:8] = bk.reshape(4, P).T
    lnw_a = np.ascontiguousarray(
        np.stack([g1, beta1 * 2048.0, g2, beta2]))

    # banded rel bias (32x): rbz[a, h, o, c] for o in 0..2; o in 3..5 zeros
    a_i = np.arange(P)[:, None]
    c_i = np.arange(P)[None, :]
    rb_hoc = np.zeros((H, 3, P, P), np.float32)
    for o in range(3):
        dist = np.minimum(np.abs((o - 1) * P + c_i - a_i), REL_MAX)
        rb_hoc[:, o] = relb[:, dist] - relb[:, REL_MAX][:, None, None]
    rbz_a = np.zeros((P, H, 6, P), np.float32)
    rbz_a[:, :, 0:3, :] = 32.0 * rb_hoc.transpose(2, 0, 1, 3)
    rbz_a = f8(rbz_a)

    expd_a = np.zeros((4, 2, P), np.float32)
    for j in range(2):
        expd_a[j, j, 0:64] = 1.0
        expd_a[2 + j, j, 64:P] = 1.0
    expd_a = np.ascontiguousarray(expd_a.reshape(4, 2 * P))

    ident = np.eye(P, dtype=np.float32)
    shared = dict(
        wq8=wq_a, wk8=wk_a, wv8=wv_a, wo8=wo_a, w18=w1_a, w28=w2_a,
        bf1p=bf1p_a, qkb=qkb_a, lnw=lnw_a, rbz8=rbz_a,
        i32d=f8(ident * (2.0 ** -5)), identfd=ident,
        identbd=ident.astype(ml_dtypes.bfloat16),
        expd=expd_a, uvec4=np.ascontiguousarray(
            np.eye(4, dtype=np.float32).reshape(1, 16)),
        ones_v8=f8(np.concatenate(
            [np.ones((P, 8, 8, 1), np.float32),
             np.zeros((P, 8, 8, 15), np.float32)], axis=3)),
        zpad=np.zeros((64, 4, U), F8NP),
    )
    if bv_nz:
        shared["bvp"] = np.ascontiguousarray((32.0 * bv)[None, :])
    if bf2_nz:
        shared["bf2p"] = np.ascontiguousarray((2048.0 * bf2)[None, :])
    if not fast_gates:
        sid_a = np.zeros((P, H, P), np.float32)
        gid_a = np.zeros((P, H, 2, P), np.float32)
        for h in range(H):
            sid_a[:, h, :] = sims[h] * ident
            gid_a[:, h, 0, :] = -gate[h] * ident
        shared["sid8"] = f8(sid_a)
        shared["gid8"] = f8(gid_a)

    in_maps = []
    for b in range(B):
        Xb = X[b]
        validf = mask[b].astype(np.float32)
        norm = np.linalg.norm(Xb, axis=-1)
        rn = (1.0 / np.maximum(norm, 1e-6)) * validf
        Pmat = np.zeros((U, NCAT), np.float32)
        Pmat[np.arange(U), np.clip(spk[b], 0, NCAT - 1)] = 1.0
        ptb_a = np.zeros((NCAT, 2, U), np.float32)
        ptb_a[:, 0, :] = 16.0 * Pmat.T
        pta_a = np.zeros((NCAT, 2, U), np.float32)
        pta_a[:, 0, :] = (-16.0 * gate[0]) * Pmat.T
        m = dict(
            xt8=f8(Xb.T.reshape(4, P, U)),
            xpbo=np.ascontiguousarray(
                (1024.0 * (Xb + bo)).reshape(8, P, D)).astype(
                    ml_dtypes.bfloat16),
            rns_b=np.ascontiguousarray((16.0 * rn)[None, :]),
            pta2=f8(pta_a),
            ptb2=f8(ptb_a),
            validd=np.ascontiguousarray(validf.reshape(8, P).T),
            **shared,
        )
        if fast_gates:
            m["rns_a"] = np.ascontiguousarray(
                (16.0 * sims[0] * rn)[None, :])
        in_maps.append(m)

    res = run_bass_kernel_spmd(nc, in_maps, core_ids=list(range(NCORES)))
    outs = [r["out"].reshape(U, D) for r in res.results]
    return np.stack(outs).astype(np.float32)
